# revision 22
# baseline (speedup 1.0000x reference)
"""Multi-head attention (RoPE, non-causal) forward on 8 TRN2 NeuronCores.

Sharding: tensor-parallel over heads (2 heads/core), zero on-device
collectives. Every core receives the full input activations plus its head
slice of Wq/Wk/Wv/Wo, computes q/k/v projections + RoPE + SDPA + its
row-parallel partial of the output projection, and the host reduces the 8
partial outputs (the row-parallel all-reduce, performed at unshard time).

Schedule (v2): the kernel is Scalar-bound -- 128 EXP activations of
[128,1024] at ~1.09us each (~140us) dominate, with PE close behind
(~142us after score-matmul row-tiling). The emission is a 9-slot
software pipeline over the 8 (batch, q-block) groups: in slot g, head-0
runs group g while head-1 runs group g-1. The two heads' score matmuls
(contract=64: h0 in PE rows 0-63, h1 in rows 64-127) are emitted as
adjacent pairs so the PE executes them concurrently via row tiling,
halving score-matmul time. Each head's score PSUM tile is single
buffered; the h0/h1 stagger keeps the exp stream dense: while ScalarE
exponentiates one head's scores the PE refills the other head's tile.
Projections (8 token stripes) and out-proj units are spread through the
slots as PE/vector filler sized to never delay the exp stream.

On-device layouts (per core, bf16 compute):
  xT      [1024 hid, 4096 tok]   tok = b*2048 + t  (host pre-transposed)
  qT/kT   per batch [128 feat, 2048 tok]   feat = hl*64 + d  (2 local heads)
  v       [128 tok-chunk, 2, 64 feats | 64 ones] x32 chunks
  scoresT [128 kpos, 1024 = 2 kblocks x 512 q]  in PSUM, exp on ScalarE
  PV      pv[d|ones, q] with ones-augmented V stationary -> row 64 = sum
  out     [1024 emb, 4096 tok]   bf16 partial of y^T (no biases)

PE warm-up runs on a memset tile starting in the NEFF preamble window
(before any DMA lands) so the HAM un-throttles to 2.4GHz by the time
real data arrives. Softmax normalization broadcasts the reciprocal row
across 64 partitions with a contract-1 PE matmul (ones[1,64].T @ rec)
instead of the slower GpSimd partition_broadcast.

Biases are separable and exact on host: bq/bk are applied on device
(per-partition add at PSUM eviction); bv contributes Wo@bv to y (softmax
rows sum to 1) and bo is additive -- both added during the host reduce.
"""

import functools
import os

import numpy as np
import ml_dtypes

_PAIR = os.environ.get("K_PAIR", "1") == "1"

B, S, HID = 2, 2048, 1024
NH, HD = 16, 64
MAX_SEQ = 65536
NCORES = 8
TOK = B * S  # 4096

_BF16 = ml_dtypes.bfloat16


def _build_graph():
    import concourse.bass as bass
    import concourse.mybir as mybir
    import concourse.tile as tile
    from concourse import bacc

    f32 = mybir.dt.float32
    bf16 = mybir.dt.bfloat16

    nc = bacc.Bacc(
        "TRN2", target_bir_lowering=False, debug=False, num_devices=NCORES
    )

    xT = nc.dram_tensor("xT", [HID, TOK], bf16, kind="ExternalInput")
    # wq/wk/wv pre-packed host-side as [128 part, 8 chunk, 128 col] so each
    # DMA descriptor moves a 2KB-contiguous partition row (4x fewer
    # descriptors than the [HID, 128] layout's 256B rows).
    wqT = nc.dram_tensor("wqT", [128, 8, 128], bf16, kind="ExternalInput")
    wkT = nc.dram_tensor("wkT", [128, 8, 128], bf16, kind="ExternalInput")
    wvT = nc.dram_tensor("wvT", [128, 8, 128], bf16, kind="ExternalInput")
    woT = nc.dram_tensor("woT", [128, HID], bf16, kind="ExternalInput")
    bqk = nc.dram_tensor("bqk", [128, 2], f32, kind="ExternalInput")
    cosT = nc.dram_tensor("cosT", [128, S], bf16, kind="ExternalInput")
    sinT = nc.dram_tensor("sinT", [128, S], bf16, kind="ExternalInput")
    rT = nc.dram_tensor("rT", [128, 128], bf16, kind="ExternalInput")
    outp = nc.dram_tensor("out", [HID, TOK], bf16, kind="ExternalOutput")

    Exp = mybir.ActivationFunctionType.Exp

    with tile.TileContext(nc, pool_alloc_mode="queue") as tc:
        with (
            tc.tile_pool(name="const", bufs=1) as const,
            tc.tile_pool(name="persist", bufs=1) as persist,
        ):
            # ---- PE warm-up fuel: a memset tile, ready before any DMA ----
            warm_w = const.tile([128, 512], bf16)
            nc.gpsimd.memset(warm_w, 0.25)

            # ---- input DMAs in need-by order (descriptors drain FIFO on
            # the sync queue, striped across the 16 HW queues) ----
            wq_sb = const.tile([128, 8, 128], bf16)
            wk_sb = const.tile([128, 8, 128], bf16)
            wv_sb = const.tile([128, 8, 128], bf16)
            for w_sb, w_dram in ((wq_sb, wqT), (wk_sb, wkT)):
                nc.sync.dma_start(out=w_sb, in_=w_dram.ap())
            xs0_c = [
                const.tile([128, 512], bf16, tag=f"xs0_{k}", name=f"xs0_{k}")
                for k in range(8)
            ]
            for k in range(8):
                nc.sync.dma_start(
                    out=xs0_c[k],
                    in_=bass.AP(
                        tensor=xT.ap().tensor,
                        offset=k * 128 * TOK,
                        ap=[[TOK, 128], [1, 512]],
                    ),
                )
            cos_sb = const.tile([128, S], bf16)
            sin_sb = const.tile([128, S], bf16)

            def rope_chunk(c):
                for sb_t, dram_t in ((cos_sb, cosT), (sin_sb, sinT)):
                    nc.sync.dma_start(
                        out=sb_t[:, c * 512 : (c + 1) * 512],
                        in_=bass.AP(
                            tensor=dram_t.ap().tensor,
                            offset=c * 512,
                            ap=[[S, 128], [1, 512]],
                        ),
                    )

            rope_chunk(0)
            rT_sb = const.tile([128, 128], bf16)
            nc.sync.dma_start(out=rT_sb, in_=rT.ap())
            bqk_sb = const.tile([128, 2], f32)
            nc.sync.dma_start(out=bqk_sb, in_=bqk.ap())
            nc.sync.dma_start(out=wv_sb, in_=wvT.ap())
            for c in range(1, 4):
                rope_chunk(c)
            wo_sb = const.tile([128, HID], bf16)
            nc.sync.dma_start(out=wo_sb, in_=woT.ap())

            # ---- persistent SBUF state ----
            qT_s = [
                [
                    persist.tile(
                        [128, 512], bf16, tag=f"qT{b}_{q}", name=f"qT{b}_{q}"
                    )
                    for q in range(4)
                ]
                for b in range(2)
            ]
            kT_h = [
                [
                    persist.tile(
                        [128, 1024], bf16, tag=f"kT{b}_{h}", name=f"kT{b}_{h}"
                    )
                    for h in range(2)
                ]
                for b in range(2)
            ]
            outT_q = [
                [
                    persist.tile(
                        [128, 512], bf16, tag=f"oT{b}_{q}", name=f"oT{b}_{q}"
                    )
                    for q in range(4)
                ]
                for b in range(2)
            ]
            # per 128-token chunk: [tok, head, 64 feats | 64 ones]
            vt = [
                persist.tile([128, 2, 128], bf16, tag=f"vt{i}", name=f"vt{i}")
                for i in range(32)
            ]
            for i in range(32):
                nc.gpsimd.memset(vt[i][:, :, 64:128], 1.0)

            with (
                tc.tile_pool(name="ps", bufs=1, space="PSUM") as ps_pool,
                tc.tile_pool(name="probs", bufs=1) as probs_pool,
                tc.tile_pool(name="norm", bufs=2) as norm_pool,
                tc.tile_pool(name="xpool", bufs=3) as xpool,
                tc.tile_pool(name="pre", bufs=3) as pre,
                tc.tile_pool(name="ysb", bufs=2) as ysb_pool,
            ):
                # PSUM: sc0,sc1 [128,1024] bufs=1 (4 banks) + pv0,pv1
                # [128,512] bufs=1 (2 banks) + aux [128,512] bufs=2
                # (2 banks, shared by warmup/qk-proj/rope/bcast/outproj)
                def sc_tile(h):
                    return ps_pool.tile(
                        [128, 1024], f32, tag=f"sc{h}", name=f"sc{h}", bufs=1
                    )

                def pv_tile(h):
                    return ps_pool.tile(
                        [128, 512], f32, tag=f"pv{h}", name=f"pv{h}", bufs=1
                    )

                def aux_tile(name):
                    return ps_pool.tile(
                        [128, 512], f32, tag="aux", name=name, bufs=2
                    )

                # ---- HAM warm-up on the memset tile: starts in the NEFF
                # preamble window, well before any DMA data lands.  Sized
                # (~8 cold + ~18 warm MMs = ~7us) to bridge until the
                # stripe-0 inputs arrive so the PE never re-throttles. ----
                def keep_warm(n):
                    wp = aux_tile("warm")
                    for wi in range(n):
                        nc.tensor.matmul(
                            wp,
                            lhsT=warm_w[:, 0:128],
                            rhs=warm_w,
                            start=(wi == 0),
                            stop=(wi == n - 1),
                        )

                keep_warm(26)

                # ---- projections + RoPE for one 512-token stripe,
                # emitted in pieces so it can be spread across slots ----
                def stripe_pieces(s):
                    sb_, sl = divmod(s, 4)
                    if s == 0:
                        xc = lambda kc: xs0_c[kc][:, :]
                    else:
                        xs = xpool.tile(
                            [128, 8, 512], bf16, tag="x", name="xs"
                        )
                        nc.sync.dma_start(
                            out=xs,
                            in_=bass.AP(
                                tensor=xT.ap().tensor,
                                offset=s * 512,
                                ap=[[TOK, 128], [TOK * 128, 8], [1, 512]],
                            ),
                        )
                        xc = lambda kc: xs[:, kc, :]
                    pcol = sl * 512
                    pieces = []

                    def qk_proj(w_sb, bias_col, dest):
                        def go():
                            ps = aux_tile("ps")
                            for kc in range(8):
                                nc.tensor.matmul(
                                    ps,
                                    lhsT=w_sb[:, kc, :],
                                    rhs=xc(kc),
                                    start=(kc == 0),
                                    stop=(kc == 7),
                                )
                            pre_sb = pre.tile(
                                [128, 512], bf16, tag="pre", name="pre_sb"
                            )
                            nc.vector.tensor_scalar_add(
                                pre_sb, ps, bqk_sb[:, bias_col : bias_col + 1]
                            )
                            rq = aux_tile("rq")
                            nc.tensor.matmul(
                                rq, lhsT=rT_sb, rhs=pre_sb, start=True,
                                stop=True,
                            )
                            t1 = pre.tile([128, 512], f32, tag="t1", name="t1")
                            nc.vector.tensor_mul(
                                t1, pre_sb, cos_sb[:, pcol : pcol + 512]
                            )
                            t2 = pre.tile([128, 512], f32, tag="t2", name="t2")
                            nc.vector.tensor_mul(
                                t2, rq, sin_sb[:, pcol : pcol + 512]
                            )
                            nc.vector.tensor_add(dest, t1, t2)

                        return go

                    pieces.append(qk_proj(wq_sb, 0, qT_s[sb_][sl][:, :]))
                    pieces.append(
                        qk_proj(
                            wk_sb,
                            1,
                            kT_h[sb_][sl // 2][
                                :, (sl % 2) * 512 : (sl % 2) * 512 + 512
                            ],
                        )
                    )

                    def v_piece(t4a, t4b):
                        def go():
                            for t4 in (t4a, t4b):
                                vp = aux_tile("vp")
                                for kc in range(8):
                                    nc.tensor.matmul(
                                        vp[:, 0:128],
                                        lhsT=xc(kc)[
                                            :, t4 * 128 : (t4 + 1) * 128
                                        ],
                                        rhs=wv_sb[:, kc, :],
                                        start=(kc == 0),
                                        stop=(kc == 7),
                                    )
                                nc.vector.tensor_copy(
                                    vt[s * 4 + t4][:, :, 0:64],
                                    vp[:, 0:128].rearrange(
                                        "p (a d) -> p a d", a=2
                                    ),
                                )

                        return go

                    pieces.append(v_piece(0, 1))
                    pieces.append(v_piece(2, 3))
                    return pieces

                # ---- SDPA building blocks ----
                NGRP = 8  # (b, qs) groups in order b*4 + qs

                def g_bq(g):
                    return g // 4, g % 4

                sc_live = {}  # h -> psum tile of scores awaiting exp
                pr_live = {}  # (h, t%2) -> probs tile
                pv_live = {}  # h -> psum accumulation tile

                def score_mm(h, g, t, i2):
                    b, qs = g_bq(g)
                    hs = slice(h * 64, (h + 1) * 64)
                    kb = 2 * t + i2
                    kcol = (kb % 8) * 128
                    if i2 == 0:
                        sc_live[h] = sc_tile(h)
                    nc.tensor.matmul(
                        sc_live[h][:, i2 * 512 : (i2 + 1) * 512],
                        lhsT=kT_h[b][kb // 8][hs, kcol : kcol + 128],
                        rhs=qT_s[b][qs][hs, :],
                        start=True,
                        stop=True,
                    )

                def exp_emit(h, t):
                    pr = probs_pool.tile(
                        [128, 1024], bf16, tag=f"pr{h}_{t % 2}",
                        name=f"pr{h}", bufs=1,
                    )
                    nc.scalar.activation(pr, sc_live[h], Exp, scale=0.125)
                    pr_live[(h, t % 2)] = pr

                def pv_mms(h, g, t):
                    b, _ = g_bq(g)
                    pr = pr_live[(h, t % 2)]
                    if t == 0:
                        pv_live[h] = pv_tile(h)
                    for i2 in range(2):
                        kc = 2 * t + i2
                        nc.tensor.matmul(
                            pv_live[h],
                            lhsT=vt[b * 16 + kc][:, h, :],
                            rhs=pr[:, i2 * 512 : (i2 + 1) * 512],
                            start=(kc == 0),
                            stop=(kc == 15),
                        )

                def norm(h, g, c0=0, c1=512):
                    b, qs = g_bq(g)
                    hs = slice(h * 64, (h + 1) * 64)
                    w = c1 - c0
                    pv = pv_live[h]
                    srow = norm_pool.tile([1, 512], f32, tag="srow", name="srow")
                    nc.vector.tensor_copy(srow[:, 0:w], pv[64:65, c0:c1])
                    rec = norm_pool.tile([1, 512], f32, tag="rec", name="rec")
                    nc.vector.reciprocal_approx_fast(
                        rec[:, 0:w], srow[:, 0:w]
                    )
                    bc = norm_pool.tile([64, 512], f32, tag="bc", name="bc")
                    nc.gpsimd.partition_broadcast(bc[:, 0:w], rec[:, 0:w])
                    nc.vector.tensor_mul(
                        outT_q[b][qs][hs, c0:c1], pv[0:64, c0:c1], bc[:, 0:w]
                    )

                # ---- out-proj unit: one 512-token column, 8 embed chunks,
                # emitted in pieces (2 chunks each) ----
                def outproj_pieces(g, engines=("vector", "vector")):
                    b, qs = g_bq(g)
                    yb = ysb_pool.tile([128, 8, 512], bf16, tag="yb", name="yb")

                    def piece(e0, last):
                        def go():
                            for e in (e0, e0 + 1):
                                yp = aux_tile("yp")
                                nc.tensor.matmul(
                                    yp,
                                    lhsT=wo_sb[:, e * 128 : (e + 1) * 128],
                                    rhs=outT_q[b][qs][:, :],
                                    start=True,
                                    stop=True,
                                )
                                eng = engines[e % 2]
                                if eng == "scalar":
                                    nc.scalar.copy(yb[:, e, :], yp)
                                else:
                                    nc.vector.tensor_copy(yb[:, e, :], yp)
                            if last:
                                nc.sync.dma_start(
                                    out=bass.AP(
                                        tensor=outp.ap().tensor,
                                        offset=b * S + qs * 512,
                                        ap=[
                                            [TOK, 128],
                                            [TOK * 128, 8],
                                            [1, 512],
                                        ],
                                    ),
                                    in_=yb,
                                )

                        return go

                    return [piece(e0, e0 == 6) for e0 in (0, 2, 4, 6)]

                # ---- the 9-slot pipeline ----
                # Stripe s pieces: [q, k, v01, v23].  Stripe 0 runs before
                # slot 0.  Stripes 1-3 feed slot 0's own later t-steps
                # (scores t=2/4/6 need k1/k2/k3; pv needs the v pieces), so
                # they are slot-0 filler in need-by order.  q1-q3 are
                # needed by slots 1-3 and emitted one slot ahead.  Stripes
                # 4-7 (batch 1, needed from slot 4) fill slots 1-4, and
                # out-proj units fill slots 4-8.
                st = [stripe_pieces(s) for s in range(8)]
                # stripe-0 q and k run before slot 0 (needed by its first
                # scores); stripe-0 v feeds slot-0's first pv steps and
                # leads the slot-0 filler.
                st[0][0]()
                st[0][1]()
                filler = [[] for _ in range(9)]
                # [q,k,v01,v23] indices:  0=q 1=k 2=v01 3=v23
                filler[0] = [
                    st[0][2], st[0][3], st[1][1], st[1][2], st[2][1],
                    st[1][3], st[2][2], st[3][1], st[2][3], st[3][2],
                    st[3][3], st[1][0],
                ]
                filler[1] = [st[2][0], st[4][1], st[4][0], st[4][2]]
                filler[2] = [st[3][0], st[5][1], st[4][3], st[5][0], st[5][2]]
                filler[3] = [st[6][1], st[6][0], st[5][3], st[6][2], st[7][1]]
                filler[4] = [st[7][0], st[6][3], st[7][2], st[7][3]]
                unit_slots = {4: [0, 1], 5: [2], 6: [3, 4], 7: [5], 8: [6]}

                for g in range(NGRP + 1):
                    if g in unit_slots:
                        for u in unit_slots[g]:
                            filler[g].extend(outproj_pieces(u))
                    fill = list(filler[g])
                    fi = 0
                    # spread filler across the 8 t-steps, emitting after
                    # the SDPA work of each step
                    for t in range(8):
                        if _PAIR:
                            for i2 in range(2):
                                if g < NGRP:
                                    score_mm(0, g, t, i2)
                                if g >= 1:
                                    score_mm(1, g - 1, t, i2)
                        else:
                            for i2 in range(2):
                                if g < NGRP:
                                    score_mm(0, g, t, i2)
                            for i2 in range(2):
                                if g >= 1:
                                    score_mm(1, g - 1, t, i2)
                        if g < NGRP:
                            exp_emit(0, t)
                        if g >= 1:
                            exp_emit(1, t)
                        if t >= 1:
                            if g < NGRP:
                                pv_mms(0, g, t - 1)
                            if g >= 1:
                                pv_mms(1, g - 1, t - 1)
                        want = ((t + 1) * len(fill) + 7) // 8
                        while fi < want:
                            fill[fi]()
                            fi += 1
                    if g < NGRP:
                        pv_mms(0, g, 7)
                        norm(0, g)
                    if g >= 1:
                        pv_mms(1, g - 1, 7)
                        if g < NGRP:
                            norm(1, g - 1)
                    # bridge the slot-boundary norm window so the PE's HAM
                    # activity monitor never sees an idle MID window
                    keep_warm(4)

                # tail: the last group's h1 norm and out-proj unit are
                # column-split so the second half's norm chain overlaps the
                # first half's matmuls/copies; copies alternate
                # vector/scalar (exp stream is finished by now).
                b7, qs7 = g_bq(7)
                yb7 = ysb_pool.tile([128, 8, 512], bf16, tag="yb", name="yb7")
                norm(1, 7, 0, 256)
                norm(1, 7, 256, 512)
                keep_warm(8)

                def unit7_half(c0, c1):
                    for e in range(8):
                        yp = aux_tile("yp7")
                        nc.tensor.matmul(
                            yp[:, c0:c1],
                            lhsT=wo_sb[:, e * 128 : (e + 1) * 128],
                            rhs=outT_q[b7][qs7][:, c0:c1],
                            start=True,
                            stop=True,
                        )
                        if e % 2 == 1:
                            nc.scalar.copy(yb7[:, e, c0:c1], yp[:, c0:c1])
                        else:
                            nc.vector.tensor_copy(
                                yb7[:, e, c0:c1], yp[:, c0:c1]
                            )

                unit7_half(0, 256)
                unit7_half(256, 512)
                nc.sync.dma_start(
                    out=bass.AP(
                        tensor=outp.ap().tensor,
                        offset=b7 * S + qs7 * 512,
                        ap=[[TOK, 128], [TOK * 128, 8], [1, 512]],
                    ),
                    in_=yb7,
                )

    nc.compile()
    return nc


@functools.lru_cache(maxsize=1)
def _get_graph():
    return _build_graph()


def _rope_tables():
    inv_freq = 1.0 / (
        MAX_SEQ ** (np.arange(0, HD, 2, dtype=np.float32) / HD)
    )
    t = np.arange(S, dtype=np.float32)
    freqs = np.einsum("i,j->ij", t, inv_freq)  # [S, 32]
    emb = np.concatenate([freqs, freqs], axis=-1)  # [S, 64]
    return np.cos(emb), np.sin(emb)


def _rot_matrix():
    r = np.zeros((HD, HD), dtype=np.float32)
    r[np.arange(32), np.arange(32) + 32] = -1.0
    r[np.arange(32) + 32, np.arange(32)] = 1.0
    rt = r.T  # lhsT so that out = R @ q
    return np.block(
        [[rt, np.zeros_like(rt)], [np.zeros_like(rt), rt]]
    )


def make_in_maps(input_embeds, Wq, bq, Wk, bk, Wv, bv, Wo, bo):
    x = np.ascontiguousarray(input_embeds, dtype=np.float32)
    xT = x.reshape(TOK, HID).T.astype(_BF16)  # [1024, 4096]
    cos, sin = _rope_tables()
    cosT = np.tile(cos.T, (2, 1)).astype(_BF16)  # [128, 2048]
    sinT = np.tile(sin.T, (2, 1)).astype(_BF16)
    rT = _rot_matrix().astype(_BF16)
    WqT = Wq.T.astype(_BF16)  # [hid, feat]
    WkT = Wk.T.astype(_BF16)
    WvT = Wv.T.astype(_BF16)
    WoT = Wo.T.astype(_BF16)  # [feat, emb]
    def pack_w(WT, fs):
        # [HID, 128] -> [128 part, 8 chunk, 128 col] (2KB-contiguous rows)
        return np.ascontiguousarray(
            WT[:, fs].reshape(8, 128, 128).transpose(1, 0, 2)
        )

    in_maps = []
    for c in range(NCORES):
        fs = slice(c * 128, (c + 1) * 128)
        in_maps.append(
            {
                "xT": xT,
                "wqT": pack_w(WqT, fs),
                "wkT": pack_w(WkT, fs),
                "wvT": pack_w(WvT, fs),
                "woT": np.ascontiguousarray(WoT[fs, :]),
                "bqk": np.ascontiguousarray(
                    np.stack([bq[fs], bk[fs]], axis=1).astype(np.float32)
                ),
                "cosT": cosT,
                "sinT": sinT,
                "rT": rT,
            }
        )
    return in_maps


def reduce_outputs(results, Wq, bq, Wk, bk, Wv, bv, Wo, bo):
    acc = np.zeros((HID, TOK), dtype=np.float32)
    for c in range(NCORES):
        acc += results[c]["out"].astype(np.float32)
    bias = bo.astype(np.float32) + Wo.astype(np.float32) @ bv.astype(np.float32)
    acc += bias[:, None]
    return np.ascontiguousarray(acc.T).reshape(B, S, HID)


def kernel(input_embeds, Wq, bq, Wk, bk, Wv, bv, Wo, bo):
    from concourse.bass_utils import run_bass_kernel_spmd

    nc = _get_graph()
    in_maps = make_in_maps(input_embeds, Wq, bq, Wk, bk, Wv, bv, Wo, bo)
    res = run_bass_kernel_spmd(
        nc, in_maps, core_ids=list(range(NCORES))
    )
    return reduce_outputs(res.results, Wq, bq, Wk, bk, Wv, bv, Wo, bo)


# revision 30
# speedup vs baseline: 1.0287x; 1.0287x over previous
"""Multi-head attention (RoPE, non-causal) forward on 8 TRN2 NeuronCores.

Sharding: tensor-parallel over heads (2 heads/core), zero on-device
collectives. Every core receives the full input activations plus its head
slice of Wq/Wk/Wv/Wo, computes q/k/v projections + RoPE + SDPA + its
row-parallel partial of the output projection, and the host reduces the 8
partial outputs (the row-parallel all-reduce, performed at unshard time).

Schedule (v2): the kernel is Scalar-bound -- 128 EXP activations of
[128,1024] at ~1.09us each (~140us) dominate, with PE close behind
(~142us after score-matmul row-tiling). The emission is a 9-slot
software pipeline over the 8 (batch, q-block) groups: in slot g, head-0
runs group g while head-1 runs group g-1. The two heads' score matmuls
(contract=64: h0 in PE rows 0-63, h1 in rows 64-127) are emitted as
adjacent pairs so the PE executes them concurrently via row tiling,
halving score-matmul time. Each head's score PSUM tile is single
buffered; the h0/h1 stagger keeps the exp stream dense: while ScalarE
exponentiates one head's scores the PE refills the other head's tile.
Projections (8 token stripes) and out-proj units are spread through the
slots as PE/vector filler sized to never delay the exp stream.

On-device layouts (per core, bf16 compute):
  xT      [1024 hid, 4096 tok]   tok = b*2048 + t  (host pre-transposed)
  qT/kT   per batch [128 feat, 2048 tok]   feat = hl*64 + d  (2 local heads)
  v       [128 tok-chunk, 2, 64 feats | 64 ones] x32 chunks
  scoresT [128 kpos, 1024 = 2 kblocks x 512 q]  in PSUM, exp on ScalarE
  PV      pv[d|ones, q] with ones-augmented V stationary -> row 64 = sum
  out     [1024 emb, 4096 tok]   bf16 partial of y^T (no biases)

PE warm-up runs on a memset tile starting in the NEFF preamble window
(before any DMA lands) so the HAM un-throttles to 2.4GHz by the time
real data arrives. Softmax normalization broadcasts the reciprocal row
across 64 partitions with a contract-1 PE matmul (ones[1,64].T @ rec)
instead of the slower GpSimd partition_broadcast.

Biases are separable and exact on host: bq/bk are applied on device
(per-partition add at PSUM eviction); bv contributes Wo@bv to y (softmax
rows sum to 1) and bo is additive -- both added during the host reduce.
"""

import functools
import os

import numpy as np
import ml_dtypes

_PAIR = os.environ.get("K_PAIR", "1") == "1"

B, S, HID = 2, 2048, 1024
NH, HD = 16, 64
MAX_SEQ = 65536
NCORES = 8
TOK = B * S  # 4096

_BF16 = ml_dtypes.bfloat16


def _build_graph():
    import concourse.bass as bass
    import concourse.mybir as mybir
    import concourse.tile as tile
    from concourse import bacc

    f32 = mybir.dt.float32
    bf16 = mybir.dt.bfloat16

    nc = bacc.Bacc(
        "TRN2", target_bir_lowering=False, debug=False, num_devices=NCORES
    )

    xT = nc.dram_tensor("xT", [HID, TOK], bf16, kind="ExternalInput")
    # wq+wk pre-packed host-side into ONE tensor of [128 part, 16 chunk,
    # 128 col] (2KB-contiguous rows, single dma_start: the sync engine's
    # per-dma_start descriptor generation is the startup serializer).
    wqkT = nc.dram_tensor("wqkT", [128, 16, 128], bf16, kind="ExternalInput")
    wvT = nc.dram_tensor("wvT", [128, 8, 128], bf16, kind="ExternalInput")
    woT = nc.dram_tensor("woT", [128, HID], bf16, kind="ExternalInput")
    bqk = nc.dram_tensor("bqk", [128, 2], f32, kind="ExternalInput")
    # cos and sin interleaved: [128 part, 2 (cos|sin), S]
    csT = nc.dram_tensor("csT", [128, 2, S], bf16, kind="ExternalInput")
    rT = nc.dram_tensor("rT", [128, 128], bf16, kind="ExternalInput")
    outp = nc.dram_tensor("out", [HID, TOK], bf16, kind="ExternalOutput")

    Exp = mybir.ActivationFunctionType.Exp

    with tile.TileContext(nc, pool_alloc_mode="queue") as tc:
        with (
            tc.tile_pool(name="const", bufs=1) as const,
            tc.tile_pool(name="persist", bufs=1) as persist,
        ):
            # ---- PE warm-up fuel: a memset tile, ready before any DMA ----
            warm_w = const.tile([128, 512], bf16)
            nc.gpsimd.memset(warm_w, 0.25)

            # ---- input DMAs, FEW dma_starts, in need-by order ----
            wqk_sb = const.tile([128, 16, 128], bf16)
            nc.sync.dma_start(out=wqk_sb, in_=wqkT.ap())
            wq_sb = wqk_sb[:, 0:8, :]
            wk_sb = wqk_sb[:, 8:16, :]
            xs0 = const.tile([128, 8, 512], bf16)
            nc.sync.dma_start(
                out=xs0,
                in_=bass.AP(
                    tensor=xT.ap().tensor,
                    offset=0,
                    ap=[[TOK, 128], [TOK * 128, 8], [1, 512]],
                ),
            )
            xs0_c = [xs0[:, k, :] for k in range(8)]
            cs_sb = const.tile([128, 2, S], bf16)
            nc.sync.dma_start(
                out=cs_sb[:, :, 0:512],
                in_=bass.AP(
                    tensor=csT.ap().tensor,
                    offset=0,
                    ap=[[2 * S, 128], [S, 2], [1, 512]],
                ),
            )
            cos_sb = cs_sb[:, 0, :]
            sin_sb = cs_sb[:, 1, :]
            rT_sb = const.tile([128, 128], bf16)
            nc.sync.dma_start(out=rT_sb, in_=rT.ap())
            bqk_sb = const.tile([128, 2], f32)
            nc.sync.dma_start(out=bqk_sb, in_=bqk.ap())
            wv_sb = const.tile([128, 8, 128], bf16)
            nc.sync.dma_start(out=wv_sb, in_=wvT.ap())
            nc.sync.dma_start(
                out=cs_sb[:, :, 512:S],
                in_=bass.AP(
                    tensor=csT.ap().tensor,
                    offset=512,
                    ap=[[2 * S, 128], [S, 2], [1, S - 512]],
                ),
            )
            wo_sb = const.tile([128, HID], bf16)
            nc.sync.dma_start(out=wo_sb, in_=woT.ap())

            # ---- persistent SBUF state ----
            qT_s = [
                [
                    persist.tile(
                        [128, 512], bf16, tag=f"qT{b}_{q}", name=f"qT{b}_{q}"
                    )
                    for q in range(4)
                ]
                for b in range(2)
            ]
            kT_h = [
                [
                    persist.tile(
                        [128, 1024], bf16, tag=f"kT{b}_{h}", name=f"kT{b}_{h}"
                    )
                    for h in range(2)
                ]
                for b in range(2)
            ]
            outT_q = [
                [
                    persist.tile(
                        [128, 512], bf16, tag=f"oT{b}_{q}", name=f"oT{b}_{q}"
                    )
                    for q in range(4)
                ]
                for b in range(2)
            ]
            # per 128-token chunk: [tok, head, 64 feats | 1 ones col].
            # A single ones column halves the LDWEIGHTS cost of every PV
            # matmul (65 stationary columns instead of 128); the PV output
            # row 64 is the softmax denominator.
            vt = [
                persist.tile([128, 2, 65], bf16, tag=f"vt{i}", name=f"vt{i}")
                for i in range(32)
            ]
            for i in range(32):
                nc.gpsimd.memset(vt[i][:, :, 64:65], 1.0)

            with (
                tc.tile_pool(name="ps", bufs=1, space="PSUM") as ps_pool,
                tc.tile_pool(name="probs", bufs=1) as probs_pool,
                tc.tile_pool(name="norm", bufs=2) as norm_pool,
                tc.tile_pool(name="xpool", bufs=3) as xpool,
                tc.tile_pool(name="pre", bufs=3) as pre,
                tc.tile_pool(name="ysb", bufs=2) as ysb_pool,
            ):
                # PSUM: sc0,sc1 [128,1024] bufs=1 (4 banks) + pv0,pv1
                # [128,512] bufs=1 (2 banks) + aux [128,512] bufs=2
                # (2 banks, shared by warmup/qk-proj/rope/bcast/outproj)
                def sc_tile(h):
                    return ps_pool.tile(
                        [128, 1024], f32, tag=f"sc{h}", name=f"sc{h}", bufs=1
                    )

                def pv_tile(h):
                    return ps_pool.tile(
                        [128, 512], f32, tag=f"pv{h}", name=f"pv{h}", bufs=1
                    )

                def aux_tile(name):
                    return ps_pool.tile(
                        [128, 512], f32, tag="aux", name=name, bufs=2
                    )

                # ---- HAM warm-up on the memset tile: starts in the NEFF
                # preamble window, well before any DMA data lands.  Sized
                # (~8 cold + ~18 warm MMs = ~7us) to bridge until the
                # stripe-0 inputs arrive so the PE never re-throttles. ----
                def keep_warm(n):
                    wp = aux_tile("warm")
                    for wi in range(n):
                        nc.tensor.matmul(
                            wp,
                            lhsT=warm_w[:, 0:128],
                            rhs=warm_w,
                            start=(wi == 0),
                            stop=(wi == n - 1),
                        )

                keep_warm(34)

                # ---- projections + RoPE for one 512-token stripe,
                # emitted in pieces so it can be spread across slots ----
                def stripe_pieces(s):
                    sb_, sl = divmod(s, 4)
                    if s == 0:
                        xc = lambda kc: xs0_c[kc][:, :]
                    else:
                        xs = xpool.tile(
                            [128, 8, 512], bf16, tag="x", name="xs"
                        )
                        nc.sync.dma_start(
                            out=xs,
                            in_=bass.AP(
                                tensor=xT.ap().tensor,
                                offset=s * 512,
                                ap=[[TOK, 128], [TOK * 128, 8], [1, 512]],
                            ),
                        )
                        xc = lambda kc: xs[:, kc, :]
                    pcol = sl * 512
                    pieces = []

                    def qk_proj(w_sb, bias_col, dest):
                        def go():
                            ps = aux_tile("ps")
                            for kc in range(8):
                                nc.tensor.matmul(
                                    ps,
                                    lhsT=w_sb[:, kc, :],
                                    rhs=xc(kc),
                                    start=(kc == 0),
                                    stop=(kc == 7),
                                )
                            pre_sb = pre.tile(
                                [128, 512], bf16, tag="pre", name="pre_sb"
                            )
                            nc.vector.tensor_scalar_add(
                                pre_sb, ps, bqk_sb[:, bias_col : bias_col + 1]
                            )
                            rq = aux_tile("rq")
                            nc.tensor.matmul(
                                rq, lhsT=rT_sb, rhs=pre_sb, start=True,
                                stop=True,
                            )
                            t1 = pre.tile([128, 512], f32, tag="t1", name="t1")
                            nc.vector.tensor_mul(
                                t1, pre_sb, cos_sb[:, pcol : pcol + 512]
                            )
                            t2 = pre.tile([128, 512], f32, tag="t2", name="t2")
                            nc.vector.tensor_mul(
                                t2, rq, sin_sb[:, pcol : pcol + 512]
                            )
                            nc.vector.tensor_add(dest, t1, t2)

                        return go

                    pieces.append(qk_proj(wq_sb, 0, qT_s[sb_][sl][:, :]))
                    pieces.append(
                        qk_proj(
                            wk_sb,
                            1,
                            kT_h[sb_][sl // 2][
                                :, (sl % 2) * 512 : (sl % 2) * 512 + 512
                            ],
                        )
                    )

                    def v_piece(t4a, t4b):
                        def go():
                            for t4 in (t4a, t4b):
                                vp = aux_tile("vp")
                                for kc in range(8):
                                    nc.tensor.matmul(
                                        vp[:, 0:128],
                                        lhsT=xc(kc)[
                                            :, t4 * 128 : (t4 + 1) * 128
                                        ],
                                        rhs=wv_sb[:, kc, :],
                                        start=(kc == 0),
                                        stop=(kc == 7),
                                    )
                                nc.vector.tensor_copy(
                                    vt[s * 4 + t4][:, :, 0:64],
                                    vp[:, 0:128].rearrange(
                                        "p (a d) -> p a d", a=2
                                    ),
                                )

                        return go

                    pieces.append(v_piece(0, 1))
                    pieces.append(v_piece(2, 3))
                    return pieces

                # ---- SDPA building blocks ----
                NGRP = 8  # (b, qs) groups in order b*4 + qs

                def g_bq(g):
                    return g // 4, g % 4

                sc_live = {}  # h -> psum tile of scores awaiting exp
                pr_live = {}  # (h, t%2) -> probs tile
                pv_live = {}  # h -> psum accumulation tile

                def score_mm(h, g, t, i2):
                    b, qs = g_bq(g)
                    hs = slice(h * 64, (h + 1) * 64)
                    kb = 2 * t + i2
                    kcol = (kb % 8) * 128
                    if i2 == 0:
                        sc_live[h] = sc_tile(h)
                    nc.tensor.matmul(
                        sc_live[h][:, i2 * 512 : (i2 + 1) * 512],
                        lhsT=kT_h[b][kb // 8][hs, kcol : kcol + 128],
                        rhs=qT_s[b][qs][hs, :],
                        start=True,
                        stop=True,
                    )

                def exp_emit(h, t):
                    pr = probs_pool.tile(
                        [128, 1024], bf16, tag=f"pr{h}_{t % 2}",
                        name=f"pr{h}", bufs=1,
                    )
                    nc.scalar.activation(pr, sc_live[h], Exp, scale=0.125)
                    pr_live[(h, t % 2)] = pr

                def pv_mms(h, g, t):
                    b, _ = g_bq(g)
                    pr = pr_live[(h, t % 2)]
                    if t == 0:
                        pv_live[h] = pv_tile(h)
                    for i2 in range(2):
                        kc = 2 * t + i2
                        nc.tensor.matmul(
                            pv_live[h][0:65, :],
                            lhsT=vt[b * 16 + kc][:, h, :],
                            rhs=pr[:, i2 * 512 : (i2 + 1) * 512],
                            start=(kc == 0),
                            stop=(kc == 15),
                        )

                def norm(h, g, c0=0, c1=512):
                    b, qs = g_bq(g)
                    hs = slice(h * 64, (h + 1) * 64)
                    w = c1 - c0
                    pv = pv_live[h]
                    srow = norm_pool.tile([1, 512], f32, tag="srow", name="srow")
                    nc.vector.tensor_copy(srow[:, 0:w], pv[64:65, c0:c1])
                    rec = norm_pool.tile([1, 512], f32, tag="rec", name="rec")
                    nc.vector.reciprocal_approx_fast(
                        rec[:, 0:w], srow[:, 0:w]
                    )
                    bc = norm_pool.tile([64, 512], f32, tag="bc", name="bc")
                    nc.gpsimd.partition_broadcast(bc[:, 0:w], rec[:, 0:w])
                    nc.vector.tensor_mul(
                        outT_q[b][qs][hs, c0:c1], pv[0:64, c0:c1], bc[:, 0:w]
                    )

                # ---- out-proj unit: one 512-token column, 8 embed chunks,
                # emitted in pieces (2 chunks each) ----
                def outproj_pieces(g, engines=("vector", "vector")):
                    b, qs = g_bq(g)
                    yb = ysb_pool.tile([128, 8, 512], bf16, tag="yb", name="yb")

                    def piece(e0, last):
                        def go():
                            for e in (e0, e0 + 1):
                                yp = aux_tile("yp")
                                nc.tensor.matmul(
                                    yp,
                                    lhsT=wo_sb[:, e * 128 : (e + 1) * 128],
                                    rhs=outT_q[b][qs][:, :],
                                    start=True,
                                    stop=True,
                                )
                                eng = engines[e % 2]
                                if eng == "scalar":
                                    nc.scalar.copy(yb[:, e, :], yp)
                                else:
                                    nc.vector.tensor_copy(yb[:, e, :], yp)
                            if last:
                                nc.sync.dma_start(
                                    out=bass.AP(
                                        tensor=outp.ap().tensor,
                                        offset=b * S + qs * 512,
                                        ap=[
                                            [TOK, 128],
                                            [TOK * 128, 8],
                                            [1, 512],
                                        ],
                                    ),
                                    in_=yb,
                                )

                        return go

                    return [piece(e0, e0 == 6) for e0 in (0, 2, 4, 6)]

                # ---- the 9-slot pipeline ----
                # Stripe s pieces: [q, k, v01, v23].  Stripe 0 runs before
                # slot 0.  Stripes 1-3 feed slot 0's own later t-steps
                # (scores t=2/4/6 need k1/k2/k3; pv needs the v pieces), so
                # they are slot-0 filler in need-by order.  q1-q3 are
                # needed by slots 1-3 and emitted one slot ahead.  Stripes
                # 4-7 (batch 1, needed from slot 4) fill slots 1-4, and
                # out-proj units fill slots 4-8.
                st = [stripe_pieces(s) for s in range(8)]
                # stripe-0 q and k run before slot 0 (needed by its first
                # scores); stripe-0 v feeds slot-0's first pv steps and
                # leads the slot-0 filler.
                st[0][0]()
                st[0][1]()
                filler = [[] for _ in range(9)]
                # [q,k,v01,v23] indices:  0=q 1=k 2=v01 3=v23
                filler[0] = [
                    st[0][2], st[0][3], st[1][1], st[1][2], st[2][1],
                    st[1][3], st[2][2], st[3][1], st[2][3], st[3][2],
                    st[3][3], st[1][0],
                ]
                filler[1] = [st[2][0], st[4][1], st[4][0], st[4][2]]
                filler[2] = [st[3][0], st[5][1], st[4][3], st[5][0], st[5][2]]
                filler[3] = [st[6][1], st[6][0], st[5][3], st[6][2], st[7][1]]
                filler[4] = [st[7][0], st[6][3], st[7][2], st[7][3]]
                unit_slots = {4: [0, 1], 5: [2], 6: [3, 4], 7: [5], 8: [6]}

                for g in range(NGRP + 1):
                    if g in unit_slots:
                        for u in unit_slots[g]:
                            filler[g].extend(outproj_pieces(u))
                    fill = list(filler[g])
                    fi = 0
                    # spread filler across the 8 t-steps, emitting after
                    # the SDPA work of each step
                    for t in range(8):
                        if _PAIR:
                            for i2 in range(2):
                                if g < NGRP:
                                    score_mm(0, g, t, i2)
                                if g >= 1:
                                    score_mm(1, g - 1, t, i2)
                        else:
                            for i2 in range(2):
                                if g < NGRP:
                                    score_mm(0, g, t, i2)
                            for i2 in range(2):
                                if g >= 1:
                                    score_mm(1, g - 1, t, i2)
                        if g < NGRP:
                            exp_emit(0, t)
                        if g >= 1:
                            exp_emit(1, t)
                        if t >= 1:
                            if g < NGRP:
                                pv_mms(0, g, t - 1)
                            if g >= 1:
                                pv_mms(1, g - 1, t - 1)
                        want = ((t + 1) * len(fill) + 7) // 8
                        while fi < want:
                            fill[fi]()
                            fi += 1
                    if g < NGRP:
                        pv_mms(0, g, 7)
                        norm(0, g)
                    if g >= 1:
                        pv_mms(1, g - 1, 7)
                        if g < NGRP:
                            norm(1, g - 1)

                # tail: the last group's h1 norm and out-proj unit are
                # column-split so the second half's norm chain overlaps the
                # first half's matmuls/copies; copies alternate
                # vector/scalar (exp stream is finished by now).
                b7, qs7 = g_bq(7)
                yb7 = ysb_pool.tile([128, 8, 512], bf16, tag="yb", name="yb7")
                norm(1, 7, 0, 256)
                norm(1, 7, 256, 512)
                keep_warm(14)

                def unit7_half(c0, c1):
                    for e in range(8):
                        yp = aux_tile("yp7")
                        nc.tensor.matmul(
                            yp[:, c0:c1],
                            lhsT=wo_sb[:, e * 128 : (e + 1) * 128],
                            rhs=outT_q[b7][qs7][:, c0:c1],
                            start=True,
                            stop=True,
                        )
                        if e % 2 == 1:
                            nc.scalar.copy(yb7[:, e, c0:c1], yp[:, c0:c1])
                        else:
                            nc.vector.tensor_copy(
                                yb7[:, e, c0:c1], yp[:, c0:c1]
                            )

                unit7_half(0, 256)
                unit7_half(256, 512)
                nc.sync.dma_start(
                    out=bass.AP(
                        tensor=outp.ap().tensor,
                        offset=b7 * S + qs7 * 512,
                        ap=[[TOK, 128], [TOK * 128, 8], [1, 512]],
                    ),
                    in_=yb7,
                )

    nc.compile()
    return nc


@functools.lru_cache(maxsize=1)
def _get_graph():
    return _build_graph()


def _rope_tables():
    inv_freq = 1.0 / (
        MAX_SEQ ** (np.arange(0, HD, 2, dtype=np.float32) / HD)
    )
    t = np.arange(S, dtype=np.float32)
    freqs = np.einsum("i,j->ij", t, inv_freq)  # [S, 32]
    emb = np.concatenate([freqs, freqs], axis=-1)  # [S, 64]
    return np.cos(emb), np.sin(emb)


def _rot_matrix():
    r = np.zeros((HD, HD), dtype=np.float32)
    r[np.arange(32), np.arange(32) + 32] = -1.0
    r[np.arange(32) + 32, np.arange(32)] = 1.0
    rt = r.T  # lhsT so that out = R @ q
    return np.block(
        [[rt, np.zeros_like(rt)], [np.zeros_like(rt), rt]]
    )


def make_in_maps(input_embeds, Wq, bq, Wk, bk, Wv, bv, Wo, bo):
    x = np.ascontiguousarray(input_embeds, dtype=np.float32)
    xT = x.reshape(TOK, HID).T.astype(_BF16)  # [1024, 4096]
    cos, sin = _rope_tables()
    cosT = np.tile(cos.T, (2, 1)).astype(_BF16)  # [128, 2048]
    sinT = np.tile(sin.T, (2, 1)).astype(_BF16)
    rT = _rot_matrix().astype(_BF16)
    WqT = Wq.T.astype(_BF16)  # [hid, feat]
    WkT = Wk.T.astype(_BF16)
    WvT = Wv.T.astype(_BF16)
    WoT = Wo.T.astype(_BF16)  # [feat, emb]
    def pack_w(WT, fs):
        # [HID, 128] -> [128 part, 8 chunk, 128 col] (2KB-contiguous rows)
        return WT[:, fs].reshape(8, 128, 128).transpose(1, 0, 2)

    csT = np.ascontiguousarray(np.stack([cosT, sinT], axis=1))  # [128,2,S]
    in_maps = []
    for c in range(NCORES):
        fs = slice(c * 128, (c + 1) * 128)
        in_maps.append(
            {
                "xT": xT,
                "wqkT": np.ascontiguousarray(
                    np.concatenate(
                        [pack_w(WqT, fs), pack_w(WkT, fs)], axis=1
                    )
                ),
                "wvT": np.ascontiguousarray(pack_w(WvT, fs)),
                "woT": np.ascontiguousarray(WoT[fs, :]),
                "bqk": np.ascontiguousarray(
                    np.stack([bq[fs], bk[fs]], axis=1).astype(np.float32)
                ),
                "csT": csT,
                "rT": rT,
            }
        )
    return in_maps


def reduce_outputs(results, Wq, bq, Wk, bk, Wv, bv, Wo, bo):
    acc = np.zeros((HID, TOK), dtype=np.float32)
    for c in range(NCORES):
        acc += results[c]["out"].astype(np.float32)
    bias = bo.astype(np.float32) + Wo.astype(np.float32) @ bv.astype(np.float32)
    acc += bias[:, None]
    return np.ascontiguousarray(acc.T).reshape(B, S, HID)


def kernel(input_embeds, Wq, bq, Wk, bk, Wv, bv, Wo, bo):
    from concourse.bass_utils import run_bass_kernel_spmd

    nc = _get_graph()
    in_maps = make_in_maps(input_embeds, Wq, bq, Wk, bk, Wv, bv, Wo, bo)
    res = run_bass_kernel_spmd(
        nc, in_maps, core_ids=list(range(NCORES))
    )
    return reduce_outputs(res.results, Wq, bq, Wk, bk, Wv, bv, Wo, bo)


# revision 36
# speedup vs baseline: 1.0935x; 1.0630x over previous
"""Multi-head attention (RoPE, non-causal) forward on 8 TRN2 NeuronCores.

Sharding: tensor-parallel over heads (2 heads/core), zero on-device
collectives. Every core receives the full input activations plus its head
slice of Wq/Wk/Wv/Wo, computes q/k/v projections + RoPE + SDPA + its
row-parallel partial of the output projection, and the host reduces the 8
partial outputs (the row-parallel all-reduce, performed at unshard time).

Schedule (v2): the kernel is Scalar-bound -- 128 EXP activations of
[128,1024] at ~1.09us each (~140us) dominate, with PE close behind
(~142us after score-matmul row-tiling). The emission is a 9-slot
software pipeline over the 8 (batch, q-block) groups: in slot g, head-0
runs group g while head-1 runs group g-1. The two heads' score matmuls
(contract=64: h0 in PE rows 0-63, h1 in rows 64-127) are emitted as
adjacent pairs so the PE executes them concurrently via row tiling,
halving score-matmul time. Each head's score PSUM tile is single
buffered; the h0/h1 stagger keeps the exp stream dense: while ScalarE
exponentiates one head's scores the PE refills the other head's tile.
Projections (8 token stripes) and out-proj units are spread through the
slots as PE/vector filler sized to never delay the exp stream.

On-device layouts (per core, bf16 compute):
  xT      [1024 hid, 4096 tok]   tok = b*2048 + t  (host pre-transposed)
  qT/kT   per batch [128 feat, 2048 tok]   feat = hl*64 + d  (2 local heads)
  v       [128 tok-chunk, 2, 64 feats | 64 ones] x32 chunks
  scoresT [128 kpos, 1024 = 2 kblocks x 512 q]  in PSUM, exp on ScalarE
  PV      pv[d|ones, q] with ones-augmented V stationary -> row 64 = sum
  out     [1024 emb, 4096 tok]   bf16 partial of y^T (no biases)

PE warm-up runs on a memset tile starting in the NEFF preamble window
(before any DMA lands) so the HAM un-throttles to 2.4GHz by the time
real data arrives. Softmax normalization broadcasts the reciprocal row
across 64 partitions with a contract-1 PE matmul (ones[1,64].T @ rec)
instead of the slower GpSimd partition_broadcast.

Biases are separable and exact on host: bq/bk are applied on device
(per-partition add at PSUM eviction); bv contributes Wo@bv to y (softmax
rows sum to 1) and bo is additive -- both added during the host reduce.
"""

import functools

import numpy as np
import ml_dtypes

B, S, HID = 2, 2048, 1024
NH, HD = 16, 64
MAX_SEQ = 65536
NCORES = 8
TOK = B * S  # 4096

_BF16 = ml_dtypes.bfloat16


def _build_graph():
    import concourse.bass as bass
    import concourse.mybir as mybir
    import concourse.tile as tile
    from concourse import bacc

    f32 = mybir.dt.float32
    bf16 = mybir.dt.bfloat16

    nc = bacc.Bacc(
        "TRN2", target_bir_lowering=False, debug=False, num_devices=NCORES
    )

    xT = nc.dram_tensor("xT", [HID, TOK], bf16, kind="ExternalInput")
    # wq+wk pre-packed host-side into ONE tensor of [128 part, 16 chunk,
    # 128 col] (2KB-contiguous rows, single dma_start: the sync engine's
    # per-dma_start descriptor generation is the startup serializer).
    wqkT = nc.dram_tensor("wqkT", [128, 16, 128], bf16, kind="ExternalInput")
    wvT = nc.dram_tensor("wvT", [128, 8, 128], bf16, kind="ExternalInput")
    woT = nc.dram_tensor("woT", [128, HID], bf16, kind="ExternalInput")
    bqk = nc.dram_tensor("bqk", [128, 2], f32, kind="ExternalInput")
    # cos and sin interleaved: [128 part, 2 (cos|sin), S]
    csT = nc.dram_tensor("csT", [128, 2, S], bf16, kind="ExternalInput")
    rT = nc.dram_tensor("rT", [128, 128], bf16, kind="ExternalInput")
    outp = nc.dram_tensor("out", [HID, TOK], bf16, kind="ExternalOutput")

    Exp = mybir.ActivationFunctionType.Exp

    with tile.TileContext(nc, pool_alloc_mode="queue") as tc:
        with (
            tc.tile_pool(name="const", bufs=1) as const,
            tc.tile_pool(name="persist", bufs=1) as persist,
        ):
            # ---- PE warm-up fuel: a memset tile, ready before any DMA ----
            warm_w = const.tile([128, 512], bf16)
            nc.gpsimd.memset(warm_w, 0.25)

            # ---- input DMAs, FEW dma_starts, in need-by order ----
            wqk_sb = const.tile([128, 16, 128], bf16)
            nc.sync.dma_start(out=wqk_sb, in_=wqkT.ap())
            wq_sb = wqk_sb[:, 0:8, :]
            wk_sb = wqk_sb[:, 8:16, :]
            xs0 = const.tile([128, 8, 512], bf16)
            nc.sync.dma_start(
                out=xs0,
                in_=bass.AP(
                    tensor=xT.ap().tensor,
                    offset=0,
                    ap=[[TOK, 128], [TOK * 128, 8], [1, 512]],
                ),
            )
            xs0_c = [xs0[:, k, :] for k in range(8)]
            cs_sb = const.tile([128, 2, S], bf16)
            nc.sync.dma_start(
                out=cs_sb[:, :, 0:512],
                in_=bass.AP(
                    tensor=csT.ap().tensor,
                    offset=0,
                    ap=[[2 * S, 128], [S, 2], [1, 512]],
                ),
            )
            cos_sb = cs_sb[:, 0, :]
            sin_sb = cs_sb[:, 1, :]
            rT_sb = const.tile([128, 128], bf16)
            nc.sync.dma_start(out=rT_sb, in_=rT.ap())
            bqk_sb = const.tile([128, 2], f32)
            nc.sync.dma_start(out=bqk_sb, in_=bqk.ap())
            wv_sb = const.tile([128, 8, 128], bf16)
            nc.sync.dma_start(out=wv_sb, in_=wvT.ap())
            nc.sync.dma_start(
                out=cs_sb[:, :, 512:S],
                in_=bass.AP(
                    tensor=csT.ap().tensor,
                    offset=512,
                    ap=[[2 * S, 128], [S, 2], [1, S - 512]],
                ),
            )
            wo_sb = const.tile([128, HID], bf16)
            nc.sync.dma_start(out=wo_sb, in_=woT.ap())

            # ---- persistent SBUF state ----
            qT_s = [
                [
                    persist.tile(
                        [128, 512], bf16, tag=f"qT{b}_{q}", name=f"qT{b}_{q}"
                    )
                    for q in range(4)
                ]
                for b in range(2)
            ]
            kT_h = [
                [
                    persist.tile(
                        [128, 1024], bf16, tag=f"kT{b}_{h}", name=f"kT{b}_{h}"
                    )
                    for h in range(2)
                ]
                for b in range(2)
            ]
            outT_q = [
                [
                    persist.tile(
                        [128, 512], bf16, tag=f"oT{b}_{q}", name=f"oT{b}_{q}"
                    )
                    for q in range(4)
                ]
                for b in range(2)
            ]
            # per 128-token chunk: [tok, head, 64 feats | 1 ones col].
            # A single ones column halves the LDWEIGHTS cost of every PV
            # matmul (65 stationary columns instead of 128); the PV output
            # row 64 is the softmax denominator.
            vt = [
                persist.tile([128, 2, 65], bf16, tag=f"vt{i}", name=f"vt{i}")
                for i in range(32)
            ]
            for i in range(32):
                nc.gpsimd.memset(vt[i][:, :, 64:65], 1.0)

            with (
                tc.tile_pool(name="ps", bufs=1, space="PSUM") as ps_pool,
                tc.tile_pool(name="probs", bufs=1) as probs_pool,
                tc.tile_pool(name="norm", bufs=2) as norm_pool,
                tc.tile_pool(name="xpool", bufs=3) as xpool,
                tc.tile_pool(name="pre", bufs=3) as pre,
                tc.tile_pool(name="ysb", bufs=2) as ysb_pool,
            ):
                # PSUM: sc0,sc1 [128,1024] bufs=1 (4 banks) + pv0,pv1
                # [128,512] bufs=1 (2 banks) + aux [128,512] bufs=2
                # (2 banks, shared by warmup/qk-proj/rope/bcast/outproj)
                def pv_tile(h):
                    return ps_pool.tile(
                        [128, 512], f32, tag=f"pv{h}", name=f"pv{h}", bufs=1
                    )

                def aux_tile(name):
                    return ps_pool.tile(
                        [128, 512], f32, tag="aux", name=name, bufs=2
                    )

                # ---- HAM warm-up on the memset tile: starts in the NEFF
                # preamble window, well before any DMA data lands.  Sized
                # (~8 cold + ~18 warm MMs = ~7us) to bridge until the
                # stripe-0 inputs arrive so the PE never re-throttles. ----
                def keep_warm(n):
                    wp = aux_tile("warm")
                    for wi in range(n):
                        nc.tensor.matmul(
                            wp,
                            lhsT=warm_w[:, 0:128],
                            rhs=warm_w,
                            start=(wi == 0),
                            stop=(wi == n - 1),
                        )

                keep_warm(34)

                # ---- projections + RoPE for one 512-token stripe,
                # emitted in pieces so it can be spread across slots ----
                def stripe_pieces(s):
                    sb_, sl = divmod(s, 4)
                    if s == 0:
                        xc = lambda kc: xs0_c[kc][:, :]
                    else:
                        xs = xpool.tile(
                            [128, 8, 512], bf16, tag="x", name="xs"
                        )
                        nc.sync.dma_start(
                            out=xs,
                            in_=bass.AP(
                                tensor=xT.ap().tensor,
                                offset=s * 512,
                                ap=[[TOK, 128], [TOK * 128, 8], [1, 512]],
                            ),
                        )
                        xc = lambda kc: xs[:, kc, :]
                    pcol = sl * 512
                    pieces = []

                    def qk_proj(w_sb, bias_col, dest):
                        def go():
                            ps = aux_tile("ps")
                            for kc in range(8):
                                nc.tensor.matmul(
                                    ps,
                                    lhsT=w_sb[:, kc, :],
                                    rhs=xc(kc),
                                    start=(kc == 0),
                                    stop=(kc == 7),
                                )
                            pre_sb = pre.tile(
                                [128, 512], bf16, tag="pre", name="pre_sb"
                            )
                            nc.vector.tensor_scalar_add(
                                pre_sb, ps, bqk_sb[:, bias_col : bias_col + 1]
                            )
                            rq = aux_tile("rq")
                            nc.tensor.matmul(
                                rq, lhsT=rT_sb, rhs=pre_sb, start=True,
                                stop=True,
                            )
                            t1 = pre.tile([128, 512], f32, tag="t1", name="t1")
                            nc.vector.tensor_mul(
                                t1, pre_sb, cos_sb[:, pcol : pcol + 512]
                            )
                            t2 = pre.tile([128, 512], f32, tag="t2", name="t2")
                            nc.vector.tensor_mul(
                                t2, rq, sin_sb[:, pcol : pcol + 512]
                            )
                            nc.vector.tensor_add(dest, t1, t2)

                        return go

                    pieces.append(qk_proj(wq_sb, 0, qT_s[sb_][sl][:, :]))
                    pieces.append(
                        qk_proj(
                            wk_sb,
                            1,
                            kT_h[sb_][sl // 2][
                                :, (sl % 2) * 512 : (sl % 2) * 512 + 512
                            ],
                        )
                    )

                    def v_piece(t4a, t4b):
                        def go():
                            for t4 in (t4a, t4b):
                                vp = aux_tile("vp")
                                for kc in range(8):
                                    nc.tensor.matmul(
                                        vp[:, 0:128],
                                        lhsT=xc(kc)[
                                            :, t4 * 128 : (t4 + 1) * 128
                                        ],
                                        rhs=wv_sb[:, kc, :],
                                        start=(kc == 0),
                                        stop=(kc == 7),
                                    )
                                nc.vector.tensor_copy(
                                    vt[s * 4 + t4][:, :, 0:64],
                                    vp[:, 0:128].rearrange(
                                        "p (a d) -> p a d", a=2
                                    ),
                                )

                        return go

                    pieces.append(v_piece(0, 1))
                    pieces.append(v_piece(2, 3))
                    return pieces

                # ---- SDPA building blocks (v3) ----
                # One k-block per step, BOTH heads in one [128,1024] score
                # tile (h0 -> cols 0:512, h1 -> cols 512:1024 = different
                # PSUM banks).  The tile is double buffered, so a step's
                # score matmuls WAR on the exp TWO steps back -- satisfied
                # at dispatch -- and the h0/h1 row-tiled pair (PE rows 0-63
                # vs 64-127) truly overlaps (~2x score throughput).
                NGRP = 8  # (b, qs) groups in order b*4 + qs

                def g_bq(g):
                    return g // 4, g % 4

                sc_live = {}  # t%3 -> psum tile of scores awaiting exp
                pr_live = {}  # t%3 -> probs tile
                pv_live = {}  # h -> psum accumulation tile

                def score_pair(g, t):
                    b, qs = g_bq(g)
                    kcol = (t % 8) * 128
                    sc = ps_pool.tile(
                        [128, 1024], f32, tag="sc", name="sc", bufs=2
                    )
                    sc_live[t % 3] = sc
                    for h in range(2):
                        hs = slice(h * 64, (h + 1) * 64)
                        nc.tensor.matmul(
                            sc[:, h * 512 : (h + 1) * 512],
                            lhsT=kT_h[b][t // 8][hs, kcol : kcol + 128],
                            rhs=qT_s[b][qs][hs, :],
                            start=True,
                            stop=True,
                        )

                def exp_emit(t):
                    pr = probs_pool.tile(
                        [128, 1024], bf16, tag=f"pr{t % 3}",
                        name="pr", bufs=1,
                    )
                    nc.scalar.activation(pr, sc_live[t % 3], Exp, scale=0.125)
                    pr_live[t % 3] = pr

                def pv_mms(g, t):
                    b, _ = g_bq(g)
                    pr = pr_live[t % 3]
                    if t == 0:
                        pv_live[0] = pv_tile(0)
                        pv_live[1] = pv_tile(1)
                    for h in range(2):
                        nc.tensor.matmul(
                            pv_live[h][0:65, :],
                            lhsT=vt[b * 16 + t][:, h, :],
                            rhs=pr[:, h * 512 : (h + 1) * 512],
                            start=(t == 0),
                            stop=(t == 15),
                        )

                def norm(h, g, c0=0, c1=512):
                    b, qs = g_bq(g)
                    hs = slice(h * 64, (h + 1) * 64)
                    w = c1 - c0
                    pv = pv_live[h]
                    srow = norm_pool.tile([1, 512], f32, tag="srow", name="srow")
                    nc.vector.tensor_copy(srow[:, 0:w], pv[64:65, c0:c1])
                    rec = norm_pool.tile([1, 512], f32, tag="rec", name="rec")
                    nc.vector.reciprocal_approx_fast(
                        rec[:, 0:w], srow[:, 0:w]
                    )
                    bc = norm_pool.tile([64, 512], f32, tag="bc", name="bc")
                    nc.gpsimd.partition_broadcast(bc[:, 0:w], rec[:, 0:w])
                    nc.vector.tensor_mul(
                        outT_q[b][qs][hs, c0:c1], pv[0:64, c0:c1], bc[:, 0:w]
                    )

                # ---- out-proj unit: one 512-token column, 8 embed chunks,
                # emitted in pieces (2 chunks each) ----
                def outproj_pieces(g, engines=("vector", "vector")):
                    b, qs = g_bq(g)
                    yb = ysb_pool.tile([128, 8, 512], bf16, tag="yb", name="yb")

                    def piece(e0, last):
                        def go():
                            for e in (e0, e0 + 1):
                                yp = aux_tile("yp")
                                nc.tensor.matmul(
                                    yp,
                                    lhsT=wo_sb[:, e * 128 : (e + 1) * 128],
                                    rhs=outT_q[b][qs][:, :],
                                    start=True,
                                    stop=True,
                                )
                                eng = engines[e % 2]
                                if eng == "scalar":
                                    nc.scalar.copy(yb[:, e, :], yp)
                                else:
                                    nc.vector.tensor_copy(yb[:, e, :], yp)
                            if last:
                                nc.sync.dma_start(
                                    out=bass.AP(
                                        tensor=outp.ap().tensor,
                                        offset=b * S + qs * 512,
                                        ap=[
                                            [TOK, 128],
                                            [TOK * 128, 8],
                                            [1, 512],
                                        ],
                                    ),
                                    in_=yb,
                                )

                        return go

                    return [piece(e0, e0 == 6) for e0 in (0, 2, 4, 6)]

                # ---- the 8-slot pipeline ----
                # Stripe s pieces: [q, k, v01, v23].  Stripe-0 q/k run
                # before slot 0; everything else is slot filler in
                # need-by order (slot g's scores need kT up to stripe
                # 4(b)+3 by t=12; pv needs vt[t-1]).  Out-proj unit g runs
                # in slot g+1 (its norms complete at slot-g end).
                st = [stripe_pieces(s) for s in range(8)]
                st[0][0]()
                st[0][1]()
                filler = [[] for _ in range(8)]
                # [q,k,v01,v23] indices:  0=q 1=k 2=v01 3=v23
                filler[0] = [
                    st[0][2], st[0][3], st[1][1], st[1][2], st[1][3],
                    st[2][1], st[2][2], st[2][3], st[3][1], st[3][2],
                    st[3][3], st[1][0],
                ]
                filler[1] = [st[2][0], st[4][1], st[4][2]]
                filler[2] = [st[3][0], st[5][1], st[4][3], st[5][2]]
                filler[3] = [st[4][0], st[6][1], st[5][3], st[6][2]]
                filler[4] = [
                    st[7][1], st[6][3], st[7][2], st[7][3], st[5][0],
                ]
                filler[5] = [st[6][0]]
                filler[6] = [st[7][0]]
                unit_slots = {
                    1: [0], 2: [1], 3: [2], 4: [3], 5: [4], 6: [5], 7: [6]
                }

                for g in range(NGRP):
                    if g in unit_slots:
                        for u in unit_slots[g]:
                            filler[g].extend(outproj_pieces(u))
                    fill = list(filler[g])
                    fi = 0
                    for t in range(16):
                        score_pair(g, t)
                        exp_emit(t)
                        if t >= 1:
                            pv_mms(g, t - 1)
                        want = ((t + 1) * len(fill) + 15) // 16
                        while fi < want:
                            fill[fi]()
                            fi += 1
                    pv_mms(g, 15)
                    if g < NGRP - 1:
                        norm(0, g)
                        norm(1, g)

                # tail: the last group's norms and out-proj unit are
                # column-split so the second half's norm chains overlap the
                # first half's matmuls/copies; copies alternate
                # vector/scalar (exp stream is finished by now).
                b7, qs7 = g_bq(7)
                yb7 = ysb_pool.tile([128, 8, 512], bf16, tag="yb", name="yb7")
                norm(0, 7, 0, 256)
                norm(1, 7, 0, 256)
                norm(0, 7, 256, 512)
                norm(1, 7, 256, 512)
                keep_warm(14)

                def unit7_half(c0, c1):
                    for e in range(8):
                        yp = aux_tile("yp7")
                        nc.tensor.matmul(
                            yp[:, c0:c1],
                            lhsT=wo_sb[:, e * 128 : (e + 1) * 128],
                            rhs=outT_q[b7][qs7][:, c0:c1],
                            start=True,
                            stop=True,
                        )
                        if e % 2 == 1:
                            nc.scalar.copy(yb7[:, e, c0:c1], yp[:, c0:c1])
                        else:
                            nc.vector.tensor_copy(
                                yb7[:, e, c0:c1], yp[:, c0:c1]
                            )

                unit7_half(0, 256)
                unit7_half(256, 512)
                nc.sync.dma_start(
                    out=bass.AP(
                        tensor=outp.ap().tensor,
                        offset=b7 * S + qs7 * 512,
                        ap=[[TOK, 128], [TOK * 128, 8], [1, 512]],
                    ),
                    in_=yb7,
                )

    nc.compile()
    return nc


@functools.lru_cache(maxsize=1)
def _get_graph():
    return _build_graph()


def _rope_tables():
    inv_freq = 1.0 / (
        MAX_SEQ ** (np.arange(0, HD, 2, dtype=np.float32) / HD)
    )
    t = np.arange(S, dtype=np.float32)
    freqs = np.einsum("i,j->ij", t, inv_freq)  # [S, 32]
    emb = np.concatenate([freqs, freqs], axis=-1)  # [S, 64]
    return np.cos(emb), np.sin(emb)


def _rot_matrix():
    r = np.zeros((HD, HD), dtype=np.float32)
    r[np.arange(32), np.arange(32) + 32] = -1.0
    r[np.arange(32) + 32, np.arange(32)] = 1.0
    rt = r.T  # lhsT so that out = R @ q
    return np.block(
        [[rt, np.zeros_like(rt)], [np.zeros_like(rt), rt]]
    )


def make_in_maps(input_embeds, Wq, bq, Wk, bk, Wv, bv, Wo, bo):
    x = np.ascontiguousarray(input_embeds, dtype=np.float32)
    xT = x.reshape(TOK, HID).T.astype(_BF16)  # [1024, 4096]
    cos, sin = _rope_tables()
    cosT = np.tile(cos.T, (2, 1)).astype(_BF16)  # [128, 2048]
    sinT = np.tile(sin.T, (2, 1)).astype(_BF16)
    rT = _rot_matrix().astype(_BF16)
    WqT = Wq.T.astype(_BF16)  # [hid, feat]
    WkT = Wk.T.astype(_BF16)
    WvT = Wv.T.astype(_BF16)
    WoT = Wo.T.astype(_BF16)  # [feat, emb]
    def pack_w(WT, fs):
        # [HID, 128] -> [128 part, 8 chunk, 128 col] (2KB-contiguous rows)
        return WT[:, fs].reshape(8, 128, 128).transpose(1, 0, 2)

    csT = np.ascontiguousarray(np.stack([cosT, sinT], axis=1))  # [128,2,S]
    in_maps = []
    for c in range(NCORES):
        fs = slice(c * 128, (c + 1) * 128)
        in_maps.append(
            {
                "xT": xT,
                "wqkT": np.ascontiguousarray(
                    np.concatenate(
                        [pack_w(WqT, fs), pack_w(WkT, fs)], axis=1
                    )
                ),
                "wvT": np.ascontiguousarray(pack_w(WvT, fs)),
                "woT": np.ascontiguousarray(WoT[fs, :]),
                "bqk": np.ascontiguousarray(
                    np.stack([bq[fs], bk[fs]], axis=1).astype(np.float32)
                ),
                "csT": csT,
                "rT": rT,
            }
        )
    return in_maps


def reduce_outputs(results, Wq, bq, Wk, bk, Wv, bv, Wo, bo):
    acc = np.zeros((HID, TOK), dtype=np.float32)
    for c in range(NCORES):
        acc += results[c]["out"].astype(np.float32)
    bias = bo.astype(np.float32) + Wo.astype(np.float32) @ bv.astype(np.float32)
    acc += bias[:, None]
    return np.ascontiguousarray(acc.T).reshape(B, S, HID)


def kernel(input_embeds, Wq, bq, Wk, bk, Wv, bv, Wo, bo):
    from concourse.bass_utils import run_bass_kernel_spmd

    nc = _get_graph()
    in_maps = make_in_maps(input_embeds, Wq, bq, Wk, bk, Wv, bv, Wo, bo)
    res = run_bass_kernel_spmd(
        nc, in_maps, core_ids=list(range(NCORES))
    )
    return reduce_outputs(res.results, Wq, bq, Wk, bk, Wv, bv, Wo, bo)


# revision 40
# speedup vs baseline: 1.1079x; 1.0131x over previous
"""Multi-head attention (RoPE, non-causal) forward on 8 TRN2 NeuronCores.

Sharding: tensor-parallel over heads (2 heads/core), zero on-device
collectives. Every core receives the full input activations plus its head
slice of Wq/Wk/Wv/Wo, computes q/k/v projections + RoPE + SDPA + its
row-parallel partial of the output projection, and the host reduces the 8
partial outputs (the row-parallel all-reduce, performed at unshard time).

Schedule (v2): the kernel is Scalar-bound -- 128 EXP activations of
[128,1024] at ~1.09us each (~140us) dominate, with PE close behind
(~142us after score-matmul row-tiling). The emission is a 9-slot
software pipeline over the 8 (batch, q-block) groups: in slot g, head-0
runs group g while head-1 runs group g-1. The two heads' score matmuls
(contract=64: h0 in PE rows 0-63, h1 in rows 64-127) are emitted as
adjacent pairs so the PE executes them concurrently via row tiling,
halving score-matmul time. Each head's score PSUM tile is single
buffered; the h0/h1 stagger keeps the exp stream dense: while ScalarE
exponentiates one head's scores the PE refills the other head's tile.
Projections (8 token stripes) and out-proj units are spread through the
slots as PE/vector filler sized to never delay the exp stream.

On-device layouts (per core, bf16 compute):
  xT      [1024 hid, 4096 tok]   tok = b*2048 + t  (host pre-transposed)
  qT/kT   per batch [128 feat, 2048 tok]   feat = hl*64 + d  (2 local heads)
  v       [128 tok-chunk, 2, 64 feats | 64 ones] x32 chunks
  scoresT [128 kpos, 1024 = 2 kblocks x 512 q]  in PSUM, exp on ScalarE
  PV      pv[d|ones, q] with ones-augmented V stationary -> row 64 = sum
  out     [1024 emb, 4096 tok]   bf16 partial of y^T (no biases)

PE warm-up runs on a memset tile starting in the NEFF preamble window
(before any DMA lands) so the HAM un-throttles to 2.4GHz by the time
real data arrives. Softmax normalization broadcasts the reciprocal row
across 64 partitions with a contract-1 PE matmul (ones[1,64].T @ rec)
instead of the slower GpSimd partition_broadcast.

Biases are separable and exact on host: bq/bk are applied on device
(per-partition add at PSUM eviction); bv contributes Wo@bv to y (softmax
rows sum to 1) and bo is additive -- both added during the host reduce.
"""

import functools

import numpy as np
import ml_dtypes

B, S, HID = 2, 2048, 1024
NH, HD = 16, 64
MAX_SEQ = 65536
NCORES = 8
TOK = B * S  # 4096

_BF16 = ml_dtypes.bfloat16


def _build_graph():
    import concourse.bass as bass
    import concourse.mybir as mybir
    import concourse.tile as tile
    from concourse import bacc

    f32 = mybir.dt.float32
    bf16 = mybir.dt.bfloat16

    nc = bacc.Bacc(
        "TRN2", target_bir_lowering=False, debug=False, num_devices=NCORES
    )

    xT = nc.dram_tensor("xT", [HID, TOK], bf16, kind="ExternalInput")
    # wq+wk pre-packed host-side into ONE tensor of [128 part, 16 chunk,
    # 128 col] (2KB-contiguous rows, single dma_start: the sync engine's
    # per-dma_start descriptor generation is the startup serializer).
    wqkT = nc.dram_tensor("wqkT", [128, 16, 128], bf16, kind="ExternalInput")
    wvT = nc.dram_tensor("wvT", [128, 8, 128], bf16, kind="ExternalInput")
    woT = nc.dram_tensor("woT", [128, HID], bf16, kind="ExternalInput")
    bqk = nc.dram_tensor("bqk", [128, 2], f32, kind="ExternalInput")
    # cos and sin interleaved: [128 part, 2 (cos|sin), S]
    csT = nc.dram_tensor("csT", [128, 2, S], bf16, kind="ExternalInput")
    rT = nc.dram_tensor("rT", [128, 128], bf16, kind="ExternalInput")
    outp = nc.dram_tensor("out", [HID, TOK], bf16, kind="ExternalOutput")

    Exp = mybir.ActivationFunctionType.Exp

    with tile.TileContext(nc, pool_alloc_mode="queue") as tc:
        with (
            tc.tile_pool(name="const", bufs=1) as const,
            tc.tile_pool(name="persist", bufs=1) as persist,
        ):
            # ---- PE warm-up fuel: a memset tile, ready before any DMA ----
            warm_w = const.tile([128, 512], bf16)
            nc.gpsimd.memset(warm_w, 0.25)

            # ---- input DMAs, FEW dma_starts, in need-by order ----
            wqk_sb = const.tile([128, 16, 128], bf16)
            nc.sync.dma_start(out=wqk_sb, in_=wqkT.ap())
            wq_sb = wqk_sb[:, 0:8, :]
            wk_sb = wqk_sb[:, 8:16, :]
            xs0 = const.tile([128, 8, 512], bf16)
            nc.sync.dma_start(
                out=xs0,
                in_=bass.AP(
                    tensor=xT.ap().tensor,
                    offset=0,
                    ap=[[TOK, 128], [TOK * 128, 8], [1, 512]],
                ),
            )
            xs0_c = [xs0[:, k, :] for k in range(8)]
            cs_sb = const.tile([128, 2, S], bf16)
            nc.sync.dma_start(
                out=cs_sb[:, :, 0:512],
                in_=bass.AP(
                    tensor=csT.ap().tensor,
                    offset=0,
                    ap=[[2 * S, 128], [S, 2], [1, 512]],
                ),
            )
            cos_sb = cs_sb[:, 0, :]
            sin_sb = cs_sb[:, 1, :]
            rT_sb = const.tile([128, 128], bf16)
            nc.sync.dma_start(out=rT_sb, in_=rT.ap())
            bqk_sb = const.tile([128, 2], f32)
            nc.sync.dma_start(out=bqk_sb, in_=bqk.ap())
            wv_sb = const.tile([128, 8, 128], bf16)
            nc.sync.dma_start(out=wv_sb, in_=wvT.ap())
            nc.sync.dma_start(
                out=cs_sb[:, :, 512:S],
                in_=bass.AP(
                    tensor=csT.ap().tensor,
                    offset=512,
                    ap=[[2 * S, 128], [S, 2], [1, S - 512]],
                ),
            )
            wo_sb = const.tile([128, HID], bf16)
            nc.sync.dma_start(out=wo_sb, in_=woT.ap())

            # ---- persistent SBUF state ----
            qT_s = [
                [
                    persist.tile(
                        [128, 512], bf16, tag=f"qT{b}_{q}", name=f"qT{b}_{q}"
                    )
                    for q in range(4)
                ]
                for b in range(2)
            ]
            kT_h = [
                [
                    persist.tile(
                        [128, 1024], bf16, tag=f"kT{b}_{h}", name=f"kT{b}_{h}"
                    )
                    for h in range(2)
                ]
                for b in range(2)
            ]
            outT_q = [
                [
                    persist.tile(
                        [128, 512], bf16, tag=f"oT{b}_{q}", name=f"oT{b}_{q}"
                    )
                    for q in range(4)
                ]
                for b in range(2)
            ]
            # per 128-token chunk: [tok, head, 64 feats | 1 ones col].
            # A single ones column halves the LDWEIGHTS cost of every PV
            # matmul (65 stationary columns instead of 128); the PV output
            # row 64 is the softmax denominator.
            vt = [
                persist.tile([128, 2, 65], bf16, tag=f"vt{i}", name=f"vt{i}")
                for i in range(32)
            ]
            for i in range(32):
                nc.gpsimd.memset(vt[i][:, :, 64:65], 1.0)

            with (
                tc.tile_pool(name="ps", bufs=1, space="PSUM") as ps_pool,
                tc.tile_pool(name="probs", bufs=1) as probs_pool,
                tc.tile_pool(name="norm", bufs=2) as norm_pool,
                tc.tile_pool(name="xpool", bufs=3) as xpool,
                tc.tile_pool(name="pre", bufs=3) as pre,
                tc.tile_pool(name="ysb", bufs=2) as ysb_pool,
            ):
                # PSUM: sc0,sc1 [128,1024] bufs=1 (4 banks) + pv0,pv1
                # [128,512] bufs=1 (2 banks) + aux [128,512] bufs=2
                # (2 banks, shared by warmup/qk-proj/rope/bcast/outproj)
                def pv_tile(h):
                    return ps_pool.tile(
                        [128, 512], f32, tag=f"pv{h}", name=f"pv{h}", bufs=1
                    )

                def aux_tile(name):
                    return ps_pool.tile(
                        [128, 512], f32, tag="aux", name=name, bufs=2
                    )

                # ---- HAM warm-up on the memset tile: starts in the NEFF
                # preamble window, well before any DMA data lands.  Sized
                # (~8 cold + ~18 warm MMs = ~7us) to bridge until the
                # stripe-0 inputs arrive so the PE never re-throttles. ----
                def keep_warm(n):
                    wp = aux_tile("warm")
                    for wi in range(n):
                        nc.tensor.matmul(
                            wp,
                            lhsT=warm_w[:, 0:128],
                            rhs=warm_w,
                            start=(wi == 0),
                            stop=(wi == n - 1),
                        )

                keep_warm(24)

                # ---- projections + RoPE for one 512-token stripe,
                # emitted in pieces so it can be spread across slots ----
                def stripe_pieces(s):
                    sb_, sl = divmod(s, 4)
                    if s == 0:
                        xc = lambda kc: xs0_c[kc][:, :]
                    else:
                        xs = xpool.tile(
                            [128, 8, 512], bf16, tag="x", name="xs"
                        )
                        nc.sync.dma_start(
                            out=xs,
                            in_=bass.AP(
                                tensor=xT.ap().tensor,
                                offset=s * 512,
                                ap=[[TOK, 128], [TOK * 128, 8], [1, 512]],
                            ),
                        )
                        xc = lambda kc: xs[:, kc, :]
                    pcol = sl * 512
                    pieces = []

                    def qk_proj(w_sb, bias_col, dest):
                        def go():
                            ps = aux_tile("ps")
                            for kc in range(8):
                                nc.tensor.matmul(
                                    ps,
                                    lhsT=w_sb[:, kc, :],
                                    rhs=xc(kc),
                                    start=(kc == 0),
                                    stop=(kc == 7),
                                )
                            pre_sb = pre.tile(
                                [128, 512], bf16, tag="pre", name="pre_sb"
                            )
                            nc.vector.tensor_scalar_add(
                                pre_sb, ps, bqk_sb[:, bias_col : bias_col + 1]
                            )
                            rq = aux_tile("rq")
                            nc.tensor.matmul(
                                rq, lhsT=rT_sb, rhs=pre_sb, start=True,
                                stop=True,
                            )
                            t1 = pre.tile([128, 512], f32, tag="t1", name="t1")
                            nc.vector.tensor_mul(
                                t1, pre_sb, cos_sb[:, pcol : pcol + 512]
                            )
                            t2 = pre.tile([128, 512], f32, tag="t2", name="t2")
                            nc.vector.tensor_mul(
                                t2, rq, sin_sb[:, pcol : pcol + 512]
                            )
                            nc.vector.tensor_add(dest, t1, t2)

                        return go

                    pieces.append(qk_proj(wq_sb, 0, qT_s[sb_][sl][:, :]))
                    pieces.append(
                        qk_proj(
                            wk_sb,
                            1,
                            kT_h[sb_][sl // 2][
                                :, (sl % 2) * 512 : (sl % 2) * 512 + 512
                            ],
                        )
                    )

                    def v_piece(t4a, t4b):
                        def go():
                            for t4 in (t4a, t4b):
                                vp = aux_tile("vp")
                                for kc in range(8):
                                    nc.tensor.matmul(
                                        vp[:, 0:128],
                                        lhsT=xc(kc)[
                                            :, t4 * 128 : (t4 + 1) * 128
                                        ],
                                        rhs=wv_sb[:, kc, :],
                                        start=(kc == 0),
                                        stop=(kc == 7),
                                    )
                                nc.vector.tensor_copy(
                                    vt[s * 4 + t4][:, :, 0:64],
                                    vp[:, 0:128].rearrange(
                                        "p (a d) -> p a d", a=2
                                    ),
                                )

                        return go

                    pieces.append(v_piece(0, 1))
                    pieces.append(v_piece(2, 3))
                    return pieces

                # ---- SDPA building blocks (v3) ----
                # One k-block per step, BOTH heads in one [128,1024] score
                # tile (h0 -> cols 0:512, h1 -> cols 512:1024 = different
                # PSUM banks).  The tile is double buffered, so a step's
                # score matmuls WAR on the exp TWO steps back -- satisfied
                # at dispatch -- and the h0/h1 row-tiled pair (PE rows 0-63
                # vs 64-127) truly overlaps (~2x score throughput).
                NGRP = 8  # (b, qs) groups in order b*4 + qs

                def g_bq(g):
                    return g // 4, g % 4

                sc_live = {}  # t%3 -> psum tile of scores awaiting exp
                pr_live = {}  # t%3 -> probs tile
                pv_live = {}  # h -> psum accumulation tile

                def score_pair(g, t):
                    b, qs = g_bq(g)
                    kcol = (t % 8) * 128
                    sc = ps_pool.tile(
                        [128, 1024], f32, tag="sc", name="sc", bufs=2
                    )
                    sc_live[t % 3] = sc
                    for h in range(2):
                        hs = slice(h * 64, (h + 1) * 64)
                        nc.tensor.matmul(
                            sc[:, h * 512 : (h + 1) * 512],
                            lhsT=kT_h[b][t // 8][hs, kcol : kcol + 128],
                            rhs=qT_s[b][qs][hs, :],
                            start=True,
                            stop=True,
                        )

                def exp_emit(t):
                    pr = probs_pool.tile(
                        [128, 1024], bf16, tag=f"pr{t % 3}",
                        name="pr", bufs=1,
                    )
                    nc.scalar.activation(pr, sc_live[t % 3], Exp, scale=0.125)
                    pr_live[t % 3] = pr

                def pv_mms(g, t):
                    b, _ = g_bq(g)
                    pr = pr_live[t % 3]
                    if t == 0:
                        pv_live[0] = pv_tile(0)
                        pv_live[1] = pv_tile(1)
                    for h in range(2):
                        nc.tensor.matmul(
                            pv_live[h][0:65, :],
                            lhsT=vt[b * 16 + t][:, h, :],
                            rhs=pr[:, h * 512 : (h + 1) * 512],
                            start=(t == 0),
                            stop=(t == 15),
                        )

                def norm(h, g, c0=0, c1=512):
                    b, qs = g_bq(g)
                    hs = slice(h * 64, (h + 1) * 64)
                    w = c1 - c0
                    pv = pv_live[h]
                    srow = norm_pool.tile([1, 512], f32, tag="srow", name="srow")
                    nc.vector.tensor_copy(srow[:, 0:w], pv[64:65, c0:c1])
                    rec = norm_pool.tile([1, 512], f32, tag="rec", name="rec")
                    nc.vector.reciprocal_approx_fast(
                        rec[:, 0:w], srow[:, 0:w]
                    )
                    bc = norm_pool.tile([64, 512], f32, tag="bc", name="bc")
                    nc.gpsimd.partition_broadcast(bc[:, 0:w], rec[:, 0:w])
                    nc.vector.tensor_mul(
                        outT_q[b][qs][hs, c0:c1], pv[0:64, c0:c1], bc[:, 0:w]
                    )

                # ---- out-proj unit: one 512-token column, 8 embed chunks,
                # emitted in pieces (2 chunks each) ----
                def outproj_pieces(g, engines=("vector", "vector")):
                    b, qs = g_bq(g)
                    yb = ysb_pool.tile([128, 8, 512], bf16, tag="yb", name="yb")

                    def piece(e0, last):
                        def go():
                            for e in (e0, e0 + 1):
                                yp = aux_tile("yp")
                                nc.tensor.matmul(
                                    yp,
                                    lhsT=wo_sb[:, e * 128 : (e + 1) * 128],
                                    rhs=outT_q[b][qs][:, :],
                                    start=True,
                                    stop=True,
                                )
                                eng = engines[e % 2]
                                if eng == "scalar":
                                    nc.scalar.copy(yb[:, e, :], yp)
                                else:
                                    nc.vector.tensor_copy(yb[:, e, :], yp)
                            if last:
                                nc.sync.dma_start(
                                    out=bass.AP(
                                        tensor=outp.ap().tensor,
                                        offset=b * S + qs * 512,
                                        ap=[
                                            [TOK, 128],
                                            [TOK * 128, 8],
                                            [1, 512],
                                        ],
                                    ),
                                    in_=yb,
                                )

                        return go

                    return [piece(e0, e0 == 6) for e0 in (0, 2, 4, 6)]

                # ---- the 8-slot pipeline ----
                # Stripe s pieces: [q, k, v01, v23].  Stripe-0 q/k run
                # before slot 0; everything else is slot filler in
                # need-by order (slot g's scores need kT up to stripe
                # 4(b)+3 by t=12; pv needs vt[t-1]).  Out-proj unit g runs
                # in slot g+1 (its norms complete at slot-g end).
                st = [stripe_pieces(s) for s in range(8)]
                st[0][0]()
                st[0][1]()
                filler = [[] for _ in range(8)]
                # [q,k,v01,v23] indices:  0=q 1=k 2=v01 3=v23
                filler[0] = [
                    st[0][2], st[0][3], st[1][1], st[1][2], st[1][3],
                    st[2][1], st[2][2], st[2][3], st[3][1], st[3][2],
                    st[3][3], st[1][0],
                ]
                filler[1] = [st[2][0], st[4][1], st[4][2]]
                filler[2] = [st[3][0], st[5][1], st[4][3], st[5][2]]
                filler[3] = [st[4][0], st[6][1], st[5][3], st[6][2]]
                filler[4] = [
                    st[7][1], st[6][3], st[7][2], st[7][3], st[5][0],
                ]
                def warm1():
                    keep_warm(1)

                filler[5] = [st[6][0], warm1, warm1, warm1]
                filler[6] = [st[7][0], warm1, warm1, warm1]
                filler[7] = [warm1, warm1, warm1, warm1]
                unit_slots = {
                    1: [0], 2: [1], 3: [2], 4: [3], 5: [4], 6: [5], 7: [6]
                }

                for g in range(NGRP):
                    if g in unit_slots:
                        for u in unit_slots[g]:
                            filler[g].extend(outproj_pieces(u))
                    fill = list(filler[g])
                    fi = 0
                    for t in range(16):
                        score_pair(g, t)
                        exp_emit(t)
                        if t >= 1:
                            pv_mms(g, t - 1)
                        want = ((t + 1) * len(fill) + 15) // 16
                        while fi < want:
                            fill[fi]()
                            fi += 1
                    pv_mms(g, 15)
                    if g < NGRP - 1:
                        norm(0, g)
                        norm(1, g)

                # tail: the last group's norms and out-proj unit are
                # column-split so the second half's norm chains overlap the
                # first half's matmuls/copies; copies alternate
                # vector/scalar (exp stream is finished by now).
                b7, qs7 = g_bq(7)
                yb7 = ysb_pool.tile([128, 8, 512], bf16, tag="yb", name="yb7")
                norm(0, 7, 0, 256)
                norm(1, 7, 0, 256)
                norm(0, 7, 256, 512)
                norm(1, 7, 256, 512)
                keep_warm(20)

                def unit7_half(c0, c1):
                    for e in range(8):
                        yp = aux_tile("yp7")
                        nc.tensor.matmul(
                            yp[:, c0:c1],
                            lhsT=wo_sb[:, e * 128 : (e + 1) * 128],
                            rhs=outT_q[b7][qs7][:, c0:c1],
                            start=True,
                            stop=True,
                        )
                        if e % 2 == 1:
                            nc.scalar.copy(yb7[:, e, c0:c1], yp[:, c0:c1])
                        else:
                            nc.vector.tensor_copy(
                                yb7[:, e, c0:c1], yp[:, c0:c1]
                            )

                unit7_half(0, 256)
                keep_warm(4)
                unit7_half(256, 512)
                nc.sync.dma_start(
                    out=bass.AP(
                        tensor=outp.ap().tensor,
                        offset=b7 * S + qs7 * 512,
                        ap=[[TOK, 128], [TOK * 128, 8], [1, 512]],
                    ),
                    in_=yb7,
                )

    nc.compile()
    return nc


@functools.lru_cache(maxsize=1)
def _get_graph():
    return _build_graph()


def _rope_tables():
    inv_freq = 1.0 / (
        MAX_SEQ ** (np.arange(0, HD, 2, dtype=np.float32) / HD)
    )
    t = np.arange(S, dtype=np.float32)
    freqs = np.einsum("i,j->ij", t, inv_freq)  # [S, 32]
    emb = np.concatenate([freqs, freqs], axis=-1)  # [S, 64]
    return np.cos(emb), np.sin(emb)


def _rot_matrix():
    r = np.zeros((HD, HD), dtype=np.float32)
    r[np.arange(32), np.arange(32) + 32] = -1.0
    r[np.arange(32) + 32, np.arange(32)] = 1.0
    rt = r.T  # lhsT so that out = R @ q
    return np.block(
        [[rt, np.zeros_like(rt)], [np.zeros_like(rt), rt]]
    )


def make_in_maps(input_embeds, Wq, bq, Wk, bk, Wv, bv, Wo, bo):
    x = np.ascontiguousarray(input_embeds, dtype=np.float32)
    xT = x.reshape(TOK, HID).T.astype(_BF16)  # [1024, 4096]
    cos, sin = _rope_tables()
    cosT = np.tile(cos.T, (2, 1)).astype(_BF16)  # [128, 2048]
    sinT = np.tile(sin.T, (2, 1)).astype(_BF16)
    rT = _rot_matrix().astype(_BF16)
    WqT = Wq.T.astype(_BF16)  # [hid, feat]
    WkT = Wk.T.astype(_BF16)
    WvT = Wv.T.astype(_BF16)
    WoT = Wo.T.astype(_BF16)  # [feat, emb]
    def pack_w(WT, fs):
        # [HID, 128] -> [128 part, 8 chunk, 128 col] (2KB-contiguous rows)
        return WT[:, fs].reshape(8, 128, 128).transpose(1, 0, 2)

    csT = np.ascontiguousarray(np.stack([cosT, sinT], axis=1))  # [128,2,S]
    in_maps = []
    for c in range(NCORES):
        fs = slice(c * 128, (c + 1) * 128)
        in_maps.append(
            {
                "xT": xT,
                "wqkT": np.ascontiguousarray(
                    np.concatenate(
                        [pack_w(WqT, fs), pack_w(WkT, fs)], axis=1
                    )
                ),
                "wvT": np.ascontiguousarray(pack_w(WvT, fs)),
                "woT": np.ascontiguousarray(WoT[fs, :]),
                "bqk": np.ascontiguousarray(
                    np.stack([bq[fs], bk[fs]], axis=1).astype(np.float32)
                ),
                "csT": csT,
                "rT": rT,
            }
        )
    return in_maps


def reduce_outputs(results, Wq, bq, Wk, bk, Wv, bv, Wo, bo):
    acc = np.zeros((HID, TOK), dtype=np.float32)
    for c in range(NCORES):
        acc += results[c]["out"].astype(np.float32)
    bias = bo.astype(np.float32) + Wo.astype(np.float32) @ bv.astype(np.float32)
    acc += bias[:, None]
    return np.ascontiguousarray(acc.T).reshape(B, S, HID)


def kernel(input_embeds, Wq, bq, Wk, bk, Wv, bv, Wo, bo):
    from concourse.bass_utils import run_bass_kernel_spmd

    nc = _get_graph()
    in_maps = make_in_maps(input_embeds, Wq, bq, Wk, bk, Wv, bv, Wo, bo)
    res = run_bass_kernel_spmd(
        nc, in_maps, core_ids=list(range(NCORES))
    )
    return reduce_outputs(res.results, Wq, bq, Wk, bk, Wv, bv, Wo, bo)


# revision 50
# speedup vs baseline: 1.1080x; 1.0001x over previous
"""Multi-head attention (RoPE, non-causal) forward on 8 TRN2 NeuronCores.

Sharding: tensor-parallel over heads (2 heads/core), zero on-device
collectives. Every core receives the full input activations plus its head
slice of Wq/Wk/Wv/Wo, computes q/k/v projections + RoPE + SDPA + its
row-parallel partial of the output projection, and the host reduces the 8
partial outputs (the row-parallel all-reduce, performed at unshard time).

Schedule (v2): the kernel is Scalar-bound -- 128 EXP activations of
[128,1024] at ~1.09us each (~140us) dominate, with PE close behind
(~142us after score-matmul row-tiling). The emission is a 9-slot
software pipeline over the 8 (batch, q-block) groups: in slot g, head-0
runs group g while head-1 runs group g-1. The two heads' score matmuls
(contract=64: h0 in PE rows 0-63, h1 in rows 64-127) are emitted as
adjacent pairs so the PE executes them concurrently via row tiling,
halving score-matmul time. Each head's score PSUM tile is single
buffered; the h0/h1 stagger keeps the exp stream dense: while ScalarE
exponentiates one head's scores the PE refills the other head's tile.
Projections (8 token stripes) and out-proj units are spread through the
slots as PE/vector filler sized to never delay the exp stream.

On-device layouts (per core, bf16 compute):
  xT      [1024 hid, 4096 tok]   tok = b*2048 + t  (host pre-transposed)
  qT/kT   per batch [128 feat, 2048 tok]   feat = hl*64 + d  (2 local heads)
  v       [128 tok-chunk, 2, 64 feats | 64 ones] x32 chunks
  scoresT [128 kpos, 1024 = 2 kblocks x 512 q]  in PSUM, exp on ScalarE
  PV      pv[d|ones, q] with ones-augmented V stationary -> row 64 = sum
  out     [1024 emb, 4096 tok]   bf16 partial of y^T (no biases)

PE warm-up runs on a memset tile starting in the NEFF preamble window
(before any DMA lands) so the HAM un-throttles to 2.4GHz by the time
real data arrives. Softmax normalization broadcasts the reciprocal row
across 64 partitions with a contract-1 PE matmul (ones[1,64].T @ rec)
instead of the slower GpSimd partition_broadcast.

Biases are separable and exact on host: bq/bk are applied on device
(per-partition add at PSUM eviction); bv contributes Wo@bv to y (softmax
rows sum to 1) and bo is additive -- both added during the host reduce.
"""

import functools

import numpy as np
import ml_dtypes

B, S, HID = 2, 2048, 1024
NH, HD = 16, 64
MAX_SEQ = 65536
NCORES = 8
TOK = B * S  # 4096

_BF16 = ml_dtypes.bfloat16


def _build_graph():
    import concourse.bass as bass
    import concourse.mybir as mybir
    import concourse.tile as tile
    from concourse import bacc

    f32 = mybir.dt.float32
    bf16 = mybir.dt.bfloat16

    nc = bacc.Bacc(
        "TRN2", target_bir_lowering=False, debug=False, num_devices=NCORES
    )

    xT = nc.dram_tensor("xT", [HID, TOK], bf16, kind="ExternalInput")
    # wq+wk pre-packed host-side into ONE tensor of [128 part, 16 chunk,
    # 128 col] (2KB-contiguous rows, single dma_start: the sync engine's
    # per-dma_start descriptor generation is the startup serializer).
    wqkT = nc.dram_tensor("wqkT", [128, 16, 128], bf16, kind="ExternalInput")
    wvT = nc.dram_tensor("wvT", [128, 8, 128], bf16, kind="ExternalInput")
    woT = nc.dram_tensor("woT", [128, HID], bf16, kind="ExternalInput")
    bqk = nc.dram_tensor("bqk", [128, 2], f32, kind="ExternalInput")
    # cos and sin interleaved: [128 part, 2 (cos|sin), S]
    csT = nc.dram_tensor("csT", [128, 2, S], bf16, kind="ExternalInput")
    rT = nc.dram_tensor("rT", [128, 128], bf16, kind="ExternalInput")
    outp = nc.dram_tensor("out", [HID, TOK], bf16, kind="ExternalOutput")

    Exp = mybir.ActivationFunctionType.Exp

    with tile.TileContext(nc, pool_alloc_mode="queue") as tc:
        with (
            tc.tile_pool(name="const", bufs=1) as const,
            tc.tile_pool(name="persist", bufs=1) as persist,
        ):
            # ---- PE warm-up fuel: a memset tile, ready before any DMA ----
            warm_w = const.tile([128, 512], bf16)
            nc.gpsimd.memset(warm_w, 0.25)

            # ---- input DMAs, FEW dma_starts, in need-by order ----
            wqk_sb = const.tile([128, 16, 128], bf16)
            nc.sync.dma_start(out=wqk_sb, in_=wqkT.ap())
            wq_sb = wqk_sb[:, 0:8, :]
            wk_sb = wqk_sb[:, 8:16, :]
            xs0 = const.tile([128, 8, 512], bf16)
            nc.sync.dma_start(
                out=xs0,
                in_=bass.AP(
                    tensor=xT.ap().tensor,
                    offset=0,
                    ap=[[TOK, 128], [TOK * 128, 8], [1, 512]],
                ),
            )
            xs0_c = [xs0[:, k, :] for k in range(8)]
            cs_sb = const.tile([128, 2, S], bf16)
            nc.sync.dma_start(
                out=cs_sb[:, :, 0:512],
                in_=bass.AP(
                    tensor=csT.ap().tensor,
                    offset=0,
                    ap=[[2 * S, 128], [S, 2], [1, 512]],
                ),
            )
            cos_sb = cs_sb[:, 0, :]
            sin_sb = cs_sb[:, 1, :]
            rT_sb = const.tile([128, 128], bf16)
            nc.sync.dma_start(out=rT_sb, in_=rT.ap())
            bqk_sb = const.tile([128, 2], f32)
            nc.sync.dma_start(out=bqk_sb, in_=bqk.ap())
            wv_sb = const.tile([128, 8, 128], bf16)
            nc.sync.dma_start(out=wv_sb, in_=wvT.ap())
            nc.sync.dma_start(
                out=cs_sb[:, :, 512:S],
                in_=bass.AP(
                    tensor=csT.ap().tensor,
                    offset=512,
                    ap=[[2 * S, 128], [S, 2], [1, S - 512]],
                ),
            )
            wo_sb = const.tile([128, HID], bf16)
            nc.sync.dma_start(out=wo_sb, in_=woT.ap())

            # ---- persistent SBUF state ----
            qT_s = [
                [
                    persist.tile(
                        [128, 512], bf16, tag=f"qT{b}_{q}", name=f"qT{b}_{q}"
                    )
                    for q in range(4)
                ]
                for b in range(2)
            ]
            kT_h = [
                [
                    persist.tile(
                        [128, 1024], bf16, tag=f"kT{b}_{h}", name=f"kT{b}_{h}"
                    )
                    for h in range(2)
                ]
                for b in range(2)
            ]
            outT_q = [
                [
                    persist.tile(
                        [128, 512], bf16, tag=f"oT{b}_{q}", name=f"oT{b}_{q}"
                    )
                    for q in range(4)
                ]
                for b in range(2)
            ]
            # per 128-token chunk: [tok, head, 64 feats | 1 ones col].
            # A single ones column halves the LDWEIGHTS cost of every PV
            # matmul (65 stationary columns instead of 128); the PV output
            # row 64 is the softmax denominator.
            vt = [
                persist.tile([128, 2, 65], bf16, tag=f"vt{i}", name=f"vt{i}")
                for i in range(32)
            ]
            for i in range(32):
                nc.gpsimd.memset(vt[i][:, :, 64:65], 1.0)

            with (
                tc.tile_pool(name="ps", bufs=1, space="PSUM") as ps_pool,
                tc.tile_pool(name="probs", bufs=1) as probs_pool,
                tc.tile_pool(name="norm", bufs=2) as norm_pool,
                tc.tile_pool(name="xpool", bufs=3) as xpool,
                tc.tile_pool(name="pre", bufs=3) as pre,
                tc.tile_pool(name="ysb", bufs=2) as ysb_pool,
            ):
                # PSUM: sc0,sc1 [128,1024] bufs=1 (4 banks) + pv0,pv1
                # [128,512] bufs=1 (2 banks) + aux [128,512] bufs=2
                # (2 banks, shared by warmup/qk-proj/rope/bcast/outproj)
                def pv_tile(h):
                    return ps_pool.tile(
                        [128, 512], f32, tag=f"pv{h}", name=f"pv{h}", bufs=1
                    )

                def aux_tile(name):
                    return ps_pool.tile(
                        [128, 512], f32, tag="aux", name=name, bufs=2
                    )

                # ---- HAM warm-up on the memset tile: starts in the NEFF
                # preamble window, well before any DMA data lands.  Sized
                # (~8 cold + ~18 warm MMs = ~7us) to bridge until the
                # stripe-0 inputs arrive so the PE never re-throttles. ----
                def keep_warm(n):
                    wp = aux_tile("warm")
                    for wi in range(n):
                        nc.tensor.matmul(
                            wp,
                            lhsT=warm_w[:, 0:128],
                            rhs=warm_w,
                            start=(wi == 0),
                            stop=(wi == n - 1),
                        )

                keep_warm(24)

                # ---- projections + RoPE for one 512-token stripe,
                # emitted in pieces so it can be spread across slots ----
                def stripe_pieces(s):
                    sb_, sl = divmod(s, 4)
                    if s == 0:
                        xc = lambda kc: xs0_c[kc][:, :]
                    else:
                        xs = xpool.tile(
                            [128, 8, 512], bf16, tag="x", name="xs"
                        )
                        nc.sync.dma_start(
                            out=xs,
                            in_=bass.AP(
                                tensor=xT.ap().tensor,
                                offset=s * 512,
                                ap=[[TOK, 128], [TOK * 128, 8], [1, 512]],
                            ),
                        )
                        xc = lambda kc: xs[:, kc, :]
                    pcol = sl * 512
                    pieces = []

                    def qk_proj(w_sb, bias_col, dest):
                        def go():
                            ps = aux_tile("ps")
                            for kc in range(8):
                                nc.tensor.matmul(
                                    ps,
                                    lhsT=w_sb[:, kc, :],
                                    rhs=xc(kc),
                                    start=(kc == 0),
                                    stop=(kc == 7),
                                )
                            pre_sb = pre.tile(
                                [128, 512], bf16, tag="pre", name="pre_sb"
                            )
                            nc.vector.tensor_scalar_add(
                                pre_sb, ps, bqk_sb[:, bias_col : bias_col + 1]
                            )
                            rq = aux_tile("rq")
                            nc.tensor.matmul(
                                rq, lhsT=rT_sb, rhs=pre_sb, start=True,
                                stop=True,
                            )
                            t1 = pre.tile([128, 512], f32, tag="t1", name="t1")
                            nc.vector.tensor_mul(
                                t1, pre_sb, cos_sb[:, pcol : pcol + 512]
                            )
                            t2 = pre.tile([128, 512], f32, tag="t2", name="t2")
                            nc.vector.tensor_mul(
                                t2, rq, sin_sb[:, pcol : pcol + 512]
                            )
                            nc.vector.tensor_add(dest, t1, t2)

                        return go

                    pieces.append(qk_proj(wq_sb, 0, qT_s[sb_][sl][:, :]))
                    pieces.append(
                        qk_proj(
                            wk_sb,
                            1,
                            kT_h[sb_][sl // 2][
                                :, (sl % 2) * 512 : (sl % 2) * 512 + 512
                            ],
                        )
                    )

                    def v_piece(t4a, t4b):
                        def go():
                            for t4 in (t4a, t4b):
                                vp = aux_tile("vp")
                                for kc in range(8):
                                    nc.tensor.matmul(
                                        vp[:, 0:128],
                                        lhsT=xc(kc)[
                                            :, t4 * 128 : (t4 + 1) * 128
                                        ],
                                        rhs=wv_sb[:, kc, :],
                                        start=(kc == 0),
                                        stop=(kc == 7),
                                    )
                                nc.vector.tensor_copy(
                                    vt[s * 4 + t4][:, :, 0:64],
                                    vp[:, 0:128].rearrange(
                                        "p (a d) -> p a d", a=2
                                    ),
                                )

                        return go

                    pieces.append(v_piece(0, 1))
                    pieces.append(v_piece(2, 3))
                    return pieces

                # ---- SDPA building blocks (v3) ----
                # One k-block per step, BOTH heads in one [128,1024] score
                # tile (h0 -> cols 0:512, h1 -> cols 512:1024 = different
                # PSUM banks).  The tile is double buffered, so a step's
                # score matmuls WAR on the exp TWO steps back -- satisfied
                # at dispatch -- and the h0/h1 row-tiled pair (PE rows 0-63
                # vs 64-127) truly overlaps (~2x score throughput).
                NGRP = 8  # (b, qs) groups in order b*4 + qs

                def g_bq(g):
                    return g // 4, g % 4

                sc_live = {}  # t%3 -> psum tile of scores awaiting exp
                pr_live = {}  # t%3 -> probs tile
                pv_live = {}  # h -> psum accumulation tile

                def score_pair(g, t):
                    b, qs = g_bq(g)
                    kcol = (t % 8) * 128
                    sc = ps_pool.tile(
                        [128, 1024], f32, tag="sc", name="sc", bufs=2
                    )
                    sc_live[t % 3] = sc
                    for h in range(2):
                        hs = slice(h * 64, (h + 1) * 64)
                        nc.tensor.matmul(
                            sc[:, h * 512 : (h + 1) * 512],
                            lhsT=kT_h[b][t // 8][hs, kcol : kcol + 128],
                            rhs=qT_s[b][qs][hs, :],
                            start=True,
                            stop=True,
                        )

                def exp_emit(t):
                    pr = probs_pool.tile(
                        [128, 1024], bf16, tag=f"pr{t % 3}",
                        name="pr", bufs=1,
                    )
                    nc.scalar.activation(pr, sc_live[t % 3], Exp, scale=0.125)
                    pr_live[t % 3] = pr

                def pv_mms(g, t):
                    b, _ = g_bq(g)
                    pr = pr_live[t % 3]
                    if t == 0:
                        pv_live[0] = pv_tile(0)
                        pv_live[1] = pv_tile(1)
                    for h in range(2):
                        nc.tensor.matmul(
                            pv_live[h][0:65, :],
                            lhsT=vt[b * 16 + t][:, h, :],
                            rhs=pr[:, h * 512 : (h + 1) * 512],
                            start=(t == 0),
                            stop=(t == 15),
                        )

                def norm(h, g, c0=0, c1=512):
                    b, qs = g_bq(g)
                    hs = slice(h * 64, (h + 1) * 64)
                    w = c1 - c0
                    pv = pv_live[h]
                    srow = norm_pool.tile([1, 512], f32, tag="srow", name="srow")
                    nc.vector.tensor_copy(srow[:, 0:w], pv[64:65, c0:c1])
                    rec = norm_pool.tile([1, 512], f32, tag="rec", name="rec")
                    nc.vector.reciprocal_approx_fast(
                        rec[:, 0:w], srow[:, 0:w]
                    )
                    bc = norm_pool.tile([64, 512], f32, tag="bc", name="bc")
                    nc.gpsimd.partition_broadcast(bc[:, 0:w], rec[:, 0:w])
                    nc.vector.tensor_mul(
                        outT_q[b][qs][hs, c0:c1], pv[0:64, c0:c1], bc[:, 0:w]
                    )

                # ---- out-proj unit: one 512-token column, 8 embed chunks,
                # emitted in pieces (2 chunks each) ----
                def outproj_pieces(g, engines=("vector", "vector")):
                    b, qs = g_bq(g)
                    yb = ysb_pool.tile([128, 8, 512], bf16, tag="yb", name="yb")

                    def piece(e0, last):
                        def go():
                            for e in (e0, e0 + 1):
                                yp = aux_tile("yp")
                                nc.tensor.matmul(
                                    yp,
                                    lhsT=wo_sb[:, e * 128 : (e + 1) * 128],
                                    rhs=outT_q[b][qs][:, :],
                                    start=True,
                                    stop=True,
                                )
                                eng = engines[e % 2]
                                if eng == "scalar":
                                    nc.scalar.copy(yb[:, e, :], yp)
                                else:
                                    nc.vector.tensor_copy(yb[:, e, :], yp)
                            if last:
                                nc.sync.dma_start(
                                    out=bass.AP(
                                        tensor=outp.ap().tensor,
                                        offset=b * S + qs * 512,
                                        ap=[
                                            [TOK, 128],
                                            [TOK * 128, 8],
                                            [1, 512],
                                        ],
                                    ),
                                    in_=yb,
                                )

                        return go

                    return [piece(e0, e0 == 6) for e0 in (0, 2, 4, 6)]

                # ---- the 8-slot pipeline ----
                # Stripe s pieces: [q, k, v01, v23].  Stripe-0 q/k run
                # before slot 0; everything else is slot filler in
                # need-by order (slot g's scores need kT up to stripe
                # 4(b)+3 by t=12; pv needs vt[t-1]).  Out-proj unit g runs
                # in slot g+1 (its norms complete at slot-g end).
                st = [stripe_pieces(s) for s in range(8)]
                st[0][0]()
                st[0][1]()
                filler = [[] for _ in range(8)]
                # [q,k,v01,v23] indices:  0=q 1=k 2=v01 3=v23
                filler[0] = [
                    st[0][2], st[0][3], st[1][1], st[1][2], st[1][3],
                    st[2][1], st[2][2], st[2][3], st[3][1], st[3][2],
                    st[3][3], st[1][0],
                ]
                filler[1] = [st[2][0], st[4][1], st[4][2]]
                filler[2] = [st[3][0], st[5][1], st[4][3], st[5][2]]
                filler[3] = [st[4][0], st[6][1], st[5][3], st[6][2]]
                filler[4] = [
                    st[7][1], st[6][3], st[7][2], st[7][3], st[5][0],
                ]
                def warm1():
                    keep_warm(1)

                filler[5] = [st[6][0], warm1, warm1, warm1]
                filler[6] = [st[7][0], warm1, warm1, warm1]
                filler[7] = [warm1, warm1, warm1, warm1]
                unit_slots = {
                    1: [0], 2: [1], 3: [2], 4: [3], 5: [4], 6: [5], 7: [6]
                }

                for g in range(NGRP):
                    if g in unit_slots:
                        for u in unit_slots[g]:
                            filler[g].extend(outproj_pieces(u))
                    fill = list(filler[g])
                    fi = 0
                    for t in range(16):
                        score_pair(g, t)
                        exp_emit(t)
                        if t >= 1:
                            pv_mms(g, t - 1)
                        want = ((t + 1) * len(fill) + 15) // 16
                        while fi < want:
                            fill[fi]()
                            fi += 1
                    pv_mms(g, 15)
                    if g < NGRP - 1:
                        norm(0, g)
                        norm(1, g)

                # tail: the last group's norms and out-proj unit are
                # column-split so the second half's norm chains overlap the
                # first half's matmuls/copies; copies alternate
                # vector/scalar (exp stream is finished by now).
                b7, qs7 = g_bq(7)
                yb7 = ysb_pool.tile([128, 8, 512], bf16, tag="yb", name="yb7")
                norm(0, 7, 0, 256)
                norm(1, 7, 0, 256)
                norm(0, 7, 256, 512)
                norm(1, 7, 256, 512)
                keep_warm(30)

                def unit7_half(c0, c1):
                    for e in range(8):
                        yp = aux_tile("yp7")
                        nc.tensor.matmul(
                            yp[:, c0:c1],
                            lhsT=wo_sb[:, e * 128 : (e + 1) * 128],
                            rhs=outT_q[b7][qs7][:, c0:c1],
                            start=True,
                            stop=True,
                        )
                        if e % 2 == 1:
                            nc.scalar.copy(yb7[:, e, c0:c1], yp[:, c0:c1])
                        else:
                            nc.vector.tensor_copy(
                                yb7[:, e, c0:c1], yp[:, c0:c1]
                            )

                unit7_half(0, 256)
                keep_warm(4)
                unit7_half(256, 512)
                nc.sync.dma_start(
                    out=bass.AP(
                        tensor=outp.ap().tensor,
                        offset=b7 * S + qs7 * 512,
                        ap=[[TOK, 128], [TOK * 128, 8], [1, 512]],
                    ),
                    in_=yb7,
                )

    nc.compile()
    return nc


@functools.lru_cache(maxsize=1)
def _get_graph():
    return _build_graph()


def _rope_tables():
    inv_freq = 1.0 / (
        MAX_SEQ ** (np.arange(0, HD, 2, dtype=np.float32) / HD)
    )
    t = np.arange(S, dtype=np.float32)
    freqs = np.einsum("i,j->ij", t, inv_freq)  # [S, 32]
    emb = np.concatenate([freqs, freqs], axis=-1)  # [S, 64]
    return np.cos(emb), np.sin(emb)


def _rot_matrix():
    r = np.zeros((HD, HD), dtype=np.float32)
    r[np.arange(32), np.arange(32) + 32] = -1.0
    r[np.arange(32) + 32, np.arange(32)] = 1.0
    rt = r.T  # lhsT so that out = R @ q
    return np.block(
        [[rt, np.zeros_like(rt)], [np.zeros_like(rt), rt]]
    )


def make_in_maps(input_embeds, Wq, bq, Wk, bk, Wv, bv, Wo, bo):
    x = np.ascontiguousarray(input_embeds, dtype=np.float32)
    xT = x.reshape(TOK, HID).T.astype(_BF16)  # [1024, 4096]
    cos, sin = _rope_tables()
    cosT = np.tile(cos.T, (2, 1)).astype(_BF16)  # [128, 2048]
    sinT = np.tile(sin.T, (2, 1)).astype(_BF16)
    rT = _rot_matrix().astype(_BF16)
    WqT = Wq.T.astype(_BF16)  # [hid, feat]
    WkT = Wk.T.astype(_BF16)
    WvT = Wv.T.astype(_BF16)
    WoT = Wo.T.astype(_BF16)  # [feat, emb]
    def pack_w(WT, fs):
        # [HID, 128] -> [128 part, 8 chunk, 128 col] (2KB-contiguous rows)
        return WT[:, fs].reshape(8, 128, 128).transpose(1, 0, 2)

    csT = np.ascontiguousarray(np.stack([cosT, sinT], axis=1))  # [128,2,S]
    in_maps = []
    for c in range(NCORES):
        fs = slice(c * 128, (c + 1) * 128)
        in_maps.append(
            {
                "xT": xT,
                "wqkT": np.ascontiguousarray(
                    np.concatenate(
                        [pack_w(WqT, fs), pack_w(WkT, fs)], axis=1
                    )
                ),
                "wvT": np.ascontiguousarray(pack_w(WvT, fs)),
                "woT": np.ascontiguousarray(WoT[fs, :]),
                "bqk": np.ascontiguousarray(
                    np.stack([bq[fs], bk[fs]], axis=1).astype(np.float32)
                ),
                "csT": csT,
                "rT": rT,
            }
        )
    return in_maps


def reduce_outputs(results, Wq, bq, Wk, bk, Wv, bv, Wo, bo):
    acc = np.zeros((HID, TOK), dtype=np.float32)
    for c in range(NCORES):
        acc += results[c]["out"].astype(np.float32)
    bias = bo.astype(np.float32) + Wo.astype(np.float32) @ bv.astype(np.float32)
    acc += bias[:, None]
    return np.ascontiguousarray(acc.T).reshape(B, S, HID)


def kernel(input_embeds, Wq, bq, Wk, bk, Wv, bv, Wo, bo):
    from concourse.bass_utils import run_bass_kernel_spmd

    nc = _get_graph()
    in_maps = make_in_maps(input_embeds, Wq, bq, Wk, bk, Wv, bv, Wo, bo)
    res = run_bass_kernel_spmd(
        nc, in_maps, core_ids=list(range(NCORES))
    )
    return reduce_outputs(res.results, Wq, bq, Wk, bk, Wv, bv, Wo, bo)
